# revision 4
# baseline (speedup 1.0000x reference)
"""Trainium2 Bass kernel for nn_ActivatedHeteroLinear (moe_routing, 8 cores).

Math: per type t in {user, item}:
    h = (x @ W1 + b1) @ W2 + b2 = x @ Wc + c        (Wc = W1@W2)
    BatchNorm (training mode) is shift-invariant -> the bias c cancels.
    out = LeakyReLU(a * u + b),  u = x @ Wc,
    a = gamma * rsqrt(var+eps),  b = beta - mean * a,
    mean/var from G = sum x x^T and S = sum_rows u (sync-BN: one ~66KB
    AllReduce of [G | S] per type).

Device pipeline per type (row tiles of [128 rows, 128 feats]):
  p1: DMA-cast x f32->bf16 in ~1MB chunks (packed: partition p holds qc
      consecutive rows). Per subtile: G += x^T x (PE, accumulating psum);
      xT = transpose(x) (PE, bf16 psum). Per 4 subtiles: evict xT (DVE);
      uT = Wc^T @ xT (PE, N=512, transposed domain: partitions = d_out);
      evict uT (ScalarE, cast bf16) into an SBUF-resident store
      [128=d_out, n_rows], accum_out collecting S.
  AR: AllReduce [G|S]; tiny on-chip stats math -> a,b as [128,1] vectors
      (per-partition in the transposed domain).
  p2: z = a*uT+b (DVE tensor_scalar w/ per-partition AP scalars);
      leaky(z) = z - min((1-slope)*z, 0) (TS + TT); PE transpose back;
      evict f32 -> staging; DMA out.

x_user [50000,64]/core is viewed host-side as row pairs [25000,128] so both
types share the d=128 path; user Wc is stacked [Wc;Wc] and uT uses two K=64
matmuls (even/odd real row of each pair); the pair-G folds its two diagonal
64x64 blocks after the AllReduce. Rows are zero-padded host-side to a
multiple of 128 (zero rows contribute nothing to G or S).
"""
import sys

for _p in ("/opt/trn_rl_repo",):
    if _p not in sys.path:
        sys.path.insert(0, _p)

import numpy as np

import concourse.mybir as mybir
import concourse.tile as tile
from concourse import bacc
from concourse.masks import make_identity
from concourse.bass_utils import run_bass_kernel_spmd

F32 = mybir.dt.float32
BF16 = mybir.dt.bfloat16
ALU = mybir.AluOpType
AFT = mybir.ActivationFunctionType

NCORES = 8
EPS = 1e-5
NEG_SLOPE = 0.01

N_USER, N_ITEM = 400000, 600000
HID, D_OUT = 256, 128

CHUNK_Q = 16   # packed subtiles per input DMA chunk
ZGRP = 8       # real 128-col blocks per pass-2 group


def _ceil_to(x, m):
    return (x + m - 1) // m * m


class TypeCfg:
    def __init__(self, name, n_rows_core, d_in, n_total_rows):
        self.name = name
        self.d_in = d_in                         # true d_in: 64 or 128
        self.paired = d_in == 64
        self.rpp = 2 if self.paired else 1       # real rows per packed row
        assert n_rows_core % self.rpp == 0
        self.Np = _ceil_to(n_rows_core // self.rpp, 128)  # padded packed rows
        self.n_rows_core = n_rows_core
        self.n_out_pad = self.Np * self.rpp
        self.N_total = n_total_rows


def _chunks(Np):
    nsub = Np // 128
    out, s = [], 0
    while s < nsub:
        qc = min(CHUNK_Q, nsub - s)
        out.append((s, qc))     # (packed-subtile index base, count)
        s += qc
    return out


def build_kernel(cfgs, ncores=NCORES):
    nc = bacc.Bacc(None, target_bir_lowering=False, num_devices=ncores)

    ext = {}
    for c in cfgs:
        ext[c.name] = {
            "x": nc.declare_dram_parameter(f"x_{c.name}", [c.Np, 128], F32, isOutput=False),
            "W1": nc.declare_dram_parameter(f"W1_{c.name}", [c.d_in, HID], F32, isOutput=False),
            "W2": nc.declare_dram_parameter(f"W2_{c.name}", [HID, D_OUT], F32, isOutput=False),
            "gamma": nc.declare_dram_parameter(f"gamma_{c.name}", [D_OUT], F32, isOutput=False),
            "beta": nc.declare_dram_parameter(f"beta_{c.name}", [D_OUT], F32, isOutput=False),
            "out": nc.declare_dram_parameter(f"out_{c.name}", [c.n_out_pad, D_OUT], F32, isOutput=True),
        }
    ar_in = {c.name: nc.dram_tensor(f"ar_in_{c.name}", [128, 129], F32) for c in cfgs}
    ar_out = {c.name: nc.dram_tensor(f"ar_out_{c.name}", [128, 129], F32, addr_space="Shared")
              for c in cfgs}
    rg = [list(range(ncores))]

    with tile.TileContext(nc) as tc:
        with (
            tc.tile_pool(name="const", bufs=1) as constp,
            tc.tile_pool(name="wsetup", bufs=1) as wsp,
            tc.tile_pool(name="wpsum", bufs=1, space="PSUM") as wps,
        ):
            ident_bf = constp.tile([128, 128], BF16)
            make_identity(nc, ident_bf[:])
            ident_f32 = constp.tile([128, 128], F32)
            make_identity(nc, ident_f32[:])
            ones_f32 = constp.tile([128, 1], F32)
            nc.gpsimd.memset(ones_f32[:], 1.0)
            eps_v = constp.tile([128, 1], F32)
            nc.gpsimd.memset(eps_v[:], EPS)
            zero_v = constp.tile([128, 1], F32)
            nc.gpsimd.memset(zero_v[:], 0.0)

            # setup: Wc = W1 @ W2 per type
            wc_f32, wc_stack = {}, {}
            for c in cfgs:
                d = c.d_in
                w1 = wsp.tile([d, HID], F32, tag=f"w1_{c.name}")
                nc.sync.dma_start(w1[:], ext[c.name]["W1"][:])
                w2a = wsp.tile([128, D_OUT], F32, tag=f"w2a_{c.name}")
                w2b = wsp.tile([128, D_OUT], F32, tag=f"w2b_{c.name}")
                nc.sync.dma_start(w2a[:], ext[c.name]["W2"][0:128, :])
                nc.sync.dma_start(w2b[:], ext[c.name]["W2"][128:256, :])
                wc_ps = wps.tile([d, D_OUT], F32, tag="wc")
                for h, w2h in enumerate((w2a, w2b)):
                    w1t_ps = wps.tile([128, d], F32, tag="w1t")
                    nc.tensor.transpose(w1t_ps[:], w1[:, h * 128:(h + 1) * 128],
                                        ident_f32[0:d, 0:d])
                    w1t = wsp.tile([128, d], F32, tag=f"w1t_{c.name}_{h}")
                    nc.vector.tensor_copy(w1t[:], w1t_ps[:])
                    nc.tensor.matmul(wc_ps[:], w1t[:], w2h[:],
                                     start=(h == 0), stop=(h == 1))
                wf = constp.tile([d, D_OUT], F32, tag=f"wcf_{c.name}")
                nc.vector.tensor_copy(wf[:], wc_ps[:])
                wc_f32[c.name] = wf
                ws = constp.tile([128, D_OUT], BF16, tag=f"wcs_{c.name}")
                nc.scalar.copy(ws[0:d, :], wc_ps[:])
                if c.paired:
                    nc.sync.dma_start(ws[64:128, :], ws[0:64, :])  # cross-part dup
                wc_stack[c.name] = ws

            for c in cfgs:
                _run_type(nc, tc, c, ext[c.name], ar_in[c.name], ar_out[c.name],
                          rg, ident_bf, ident_f32, ones_f32, eps_v, zero_v,
                          wc_f32[c.name], wc_stack[c.name])

    nc.finalize()
    return nc


def _run_type(nc, tc, c, ex, ar_in, ar_out, rg, ident_bf, ident_f32, ones_f32,
              eps_v, zero_v, wc_f32, wc_stack):
    name, d, rpp = c.name, c.d_in, c.rpp
    chunks = _chunks(c.Np)
    nsub = c.Np // 128
    n_groups = sum((qc + 3) // 4 for _, qc in chunks)
    n_acc = n_groups * (2 if c.paired else 1)

    with (
        tc.tile_pool(name=f"ut_{name}", bufs=1) as utp,
        tc.tile_pool(name=f"io_{name}", bufs=2) as iop,
        tc.tile_pool(name=f"sb_{name}", bufs=2) as sbp,
        tc.tile_pool(name=f"small_{name}", bufs=1) as smp,
        tc.tile_pool(name=f"gps_{name}", bufs=1, space="PSUM") as gps,
        tc.tile_pool(name=f"ps_{name}", bufs=2, space="PSUM") as psp,
        tc.tile_pool(name=f"sps_{name}", bufs=1, space="PSUM") as sps,
    ):
        ut_store = utp.tile([128, c.Np * rpp], BF16)   # col = real row index
        g_ps = gps.tile([128, 128], F32)
        acc = smp.tile([128, n_acc], F32)

        # ---------------- pass 1 ----------------
        acc_i = 0
        sub_i = 0
        for s0, qc in chunks:
            xch = iop.tile([128, CHUNK_Q * 128], BF16, tag="xch")
            src = ex["x"][s0 * 128:(s0 + qc) * 128, :].rearrange(
                "(p q) d -> p (q d)", p=128)
            nc.gpsimd.dma_start(xch[:, 0:qc * 128], src)   # f32 -> bf16 cast
            for g0 in range(0, qc, 4):
                ns = min(4, qc - g0)
                xt_ps = psp.tile([128, 512], BF16, tag="xtnat")
                for i in range(ns):
                    sl = xch[:, (g0 + i) * 128:(g0 + i + 1) * 128]
                    nc.tensor.matmul(g_ps[:], sl, sl,
                                     start=(sub_i == 0), stop=(sub_i == nsub - 1),
                                     skip_group_check=True)
                    sub_i += 1
                    nc.tensor.transpose(xt_ps[:, i * 128:(i + 1) * 128], sl,
                                        ident_bf[:])
                xt_sb = sbp.tile([128, 512], BF16, tag="xtsb")
                nc.vector.tensor_copy(xt_sb[:, 0:ns * 128], xt_ps[:, 0:ns * 128])
                blk0 = s0 + g0          # first packed-subtile block index
                if not c.paired:
                    ut_ps = psp.tile([128, 512], F32, tag="utps")
                    nc.tensor.matmul(ut_ps[:, 0:ns * 128], wc_stack[0:d, :],
                                     xt_sb[0:d, 0:ns * 128], start=True, stop=True)
                    nc.scalar.activation(
                        ut_store[:, blk0 * 128:(blk0 + ns) * 128],
                        ut_ps[:, 0:ns * 128],
                        AFT.Copy, accum_out=acc[:, acc_i:acc_i + 1])
                    acc_i += 1
                else:
                    for half in range(2):
                        ut_ps = psp.tile([128, 512], F32, tag="utps")
                        nc.tensor.matmul(
                            ut_ps[:, 0:ns * 128],
                            wc_stack[half * 64:(half + 1) * 64, :],
                            xt_sb[half * 64:(half + 1) * 64, 0:ns * 128],
                            start=True, stop=True)
                        # real-row col blocks 2*(blk0+i) + half, i in [0, ns)
                        dst = ut_store[:, 2 * blk0 * 128:2 * (blk0 + ns) * 128]
                        dst = dst.rearrange("p (n two f) -> p n two f",
                                            two=2, f=128)[:, :, half, :]
                        src3 = ut_ps[:, 0:ns * 128].rearrange(
                            "p (n f) -> p n f", f=128)
                        nc.scalar.activation(
                            dst, src3, AFT.Copy,
                            accum_out=acc[:, acc_i:acc_i + 1])
                        acc_i += 1
        assert acc_i == n_acc and sub_i == nsub

        # ---------------- AllReduce + stats ----------------
        g_sb = smp.tile([128, 128], F32, tag="gsb")
        nc.vector.tensor_copy(g_sb[:], g_ps[:])
        s_sb = smp.tile([128, 1], F32, tag="ssb")
        nc.vector.reduce_sum(s_sb[:], acc[:, 0:n_acc], axis=mybir.AxisListType.X)
        nc.sync.dma_start(ar_in[:, 0:128], g_sb[:])
        nc.sync.dma_start(ar_in[:, 128:129], s_sb[:])
        nc.gpsimd.collective_compute(
            "AllReduce", ALU.add, replica_groups=rg,
            ins=[ar_in[:]], outs=[ar_out[:]])
        ar_sb = smp.tile([128, 129], F32, tag="arsb")
        nc.sync.dma_start(ar_sb[:], ar_out[:])

        if c.paired:
            blk11 = smp.tile([64, 64], F32, tag="blk11")
            nc.sync.dma_start(blk11[:], ar_sb[64:128, 64:128])  # cross-partition
            g_eff = smp.tile([64, 64], F32, tag="geff")
            nc.vector.tensor_tensor(g_eff[:], ar_sb[0:64, 0:64], blk11[:], ALU.add)
            g_ap = g_eff[:]
        else:
            g_ap = ar_sb[:, 0:128]

        t1_ps = sps.tile([d, D_OUT], F32, tag="sps")
        nc.tensor.matmul(t1_ps[:], g_ap, wc_f32[:], start=True, stop=True)
        t1 = smp.tile([d, D_OUT], F32, tag="t1")
        nc.vector.tensor_copy(t1[:], t1_ps[:])
        t2 = smp.tile([d, D_OUT], F32, tag="t2")
        nc.vector.tensor_tensor(t2[:], t1[:], wc_f32[:], ALU.mult)
        e2_ps = sps.tile([1, D_OUT], F32, tag="sps")
        nc.tensor.matmul(e2_ps[:], ones_f32[0:d, :], t2[:], start=True, stop=True)
        rowmat = smp.tile([3, 128], F32, tag="rowmat")
        nc.vector.tensor_copy(rowmat[0:1, :], e2_ps[:])
        nc.sync.dma_start(rowmat[1:2, :],
                          ex["gamma"][:].rearrange("(o f) -> o f", o=1))
        nc.sync.dma_start(rowmat[2:3, :],
                          ex["beta"][:].rearrange("(o f) -> o f", o=1))
        cols_ps = sps.tile([128, 3], F32, tag="sps")
        nc.tensor.transpose(cols_ps[:], rowmat[:], ident_f32[0:3, 0:3])
        cols = smp.tile([128, 3], F32, tag="cols")
        nc.vector.tensor_copy(cols[:], cols_ps[:])

        inv_n = 1.0 / float(c.N_total)
        mean = smp.tile([128, 1], F32, tag="mean")
        nc.vector.tensor_scalar(mean[:], ar_sb[:, 128:129], inv_n, None, ALU.mult)
        msq = smp.tile([128, 1], F32, tag="msq")
        nc.vector.tensor_tensor(msq[:], mean[:], mean[:], ALU.mult)
        e2n = smp.tile([128, 1], F32, tag="e2n")
        nc.vector.tensor_scalar(e2n[:], cols[:, 0:1], inv_n, None, ALU.mult)
        var = smp.tile([128, 1], F32, tag="var")
        nc.vector.tensor_tensor(var[:], e2n[:], msq[:], ALU.subtract)
        lnv = smp.tile([128, 1], F32, tag="lnv")
        nc.scalar.activation(lnv[:], var[:], AFT.Ln, bias=eps_v[:], scale=1.0)
        rstd = smp.tile([128, 1], F32, tag="rstd")
        nc.scalar.activation(rstd[:], lnv[:], AFT.Exp, bias=zero_v[:], scale=-0.5)
        a_vec = smp.tile([128, 1], F32, tag="avec")
        nc.vector.tensor_tensor(a_vec[:], cols[:, 1:2], rstd[:], ALU.mult)
        ma = smp.tile([128, 1], F32, tag="ma")
        nc.vector.tensor_tensor(ma[:], mean[:], a_vec[:], ALU.mult)
        b_vec = smp.tile([128, 1], F32, tag="bvec")
        nc.vector.tensor_tensor(b_vec[:], cols[:, 2:3], ma[:], ALU.subtract)

        # ---------------- pass 2 ----------------
        for s0, qc in chunks:
            nblk = qc * rpp                       # real 128-col blocks
            rblk0 = s0 * rpp                      # first real block of chunk
            for z0 in range(0, nblk, ZGRP):
                zn = min(ZGRP, nblk - z0)
                zw = zn * 128
                zbuf = sbp.tile([128, ZGRP * 128], BF16, tag="zbuf")
                qbuf = sbp.tile([128, ZGRP * 128], BF16, tag="qbuf")
                wbuf = sbp.tile([128, ZGRP * 128], BF16, tag="wbuf")
                ucols = ut_store[:, (rblk0 + z0) * 128:(rblk0 + z0) * 128 + zw]
                nc.vector.tensor_scalar(zbuf[:, 0:zw], ucols, a_vec[:], b_vec[:],
                                        ALU.mult, ALU.add)
                nc.vector.tensor_scalar(qbuf[:, 0:zw], zbuf[:, 0:zw],
                                        1.0 - NEG_SLOPE, 0.0, ALU.mult, ALU.min)
                nc.vector.tensor_tensor(wbuf[:, 0:zw], zbuf[:, 0:zw],
                                        qbuf[:, 0:zw], ALU.subtract)
                stg = iop.tile([128, ZGRP * 128], F32, tag="stg")
                for t0 in range(0, zn, 4):
                    tn = min(4, zn - t0)
                    nat_ps = psp.tile([128, 512], BF16, tag="xtnat")
                    for i in range(tn):
                        nc.tensor.transpose(
                            nat_ps[:, i * 128:(i + 1) * 128],
                            wbuf[:, (t0 + i) * 128:(t0 + i + 1) * 128],
                            ident_bf[:])
                    nc.scalar.copy(stg[:, t0 * 128:t0 * 128 + tn * 128],
                                   nat_ps[:, 0:tn * 128])
                dst = ex["out"][s0 * 128 * rpp:(s0 + qc) * 128 * rpp, :].rearrange(
                    "(p q) e -> p (q e)", p=128)[:, z0 * 128:z0 * 128 + zw]
                nc.sync.dma_start(dst, stg[:, 0:zw])


# ---------------------------------------------------------------------------
_BUILT = {}


def _get_built():
    if "full" not in _BUILT:
        user = TypeCfg("user", N_USER // NCORES, 64, N_USER)
        item = TypeCfg("item", N_ITEM // NCORES, 128, N_ITEM)
        _BUILT["full"] = (build_kernel([user, item]), user, item)
    return _BUILT["full"]


def kernel(x_user, x_item,
           W1_user=None, b1_user=None, W1_item=None, b1_item=None,
           W2_user=None, b2_user=None, W2_item=None, b2_item=None,
           gamma_user=None, beta_user=None, gamma_item=None, beta_item=None,
           _trace=False):
    nc, ucfg, icfg = _get_built()

    def prep(x, cfg):
        x = np.ascontiguousarray(np.asarray(x, np.float32))
        n = x.shape[0] // NCORES
        shards = []
        for i in range(NCORES):
            s = x[i * n:(i + 1) * n].reshape(-1, 128)
            pad = cfg.Np - s.shape[0]
            if pad:
                s = np.concatenate([s, np.zeros((pad, 128), np.float32)], 0)
            shards.append(s)
        return shards

    xu = prep(x_user, ucfg)
    xi = prep(x_item, icfg)
    common = {
        "W1_user": np.asarray(W1_user, np.float32),
        "W2_user": np.asarray(W2_user, np.float32),
        "gamma_user": np.asarray(gamma_user, np.float32),
        "beta_user": np.asarray(beta_user, np.float32),
        "W1_item": np.asarray(W1_item, np.float32),
        "W2_item": np.asarray(W2_item, np.float32),
        "gamma_item": np.asarray(gamma_item, np.float32),
        "beta_item": np.asarray(beta_item, np.float32),
    }
    in_maps = [dict(common, x_user=xu[i], x_item=xi[i]) for i in range(NCORES)]
    res = run_bass_kernel_spmd(nc, in_maps, list(range(NCORES)), trace=_trace)
    nu, ni = N_USER // NCORES, N_ITEM // NCORES
    out_user = np.concatenate(
        [res.results[i]["out_user"][:nu] for i in range(NCORES)], 0)
    out_item = np.concatenate(
        [res.results[i]["out_item"][:ni] for i in range(NCORES)], 0)
    if _trace:
        kernel.last_exec_time_ns = res.exec_time_ns
    return (out_user, out_item)


# revision 15
# speedup vs baseline: 1.4398x; 1.4398x over previous
"""Trainium2 Bass kernel for nn_ActivatedHeteroLinear (moe_routing, 8 cores).

Math: per type t in {user, item}:
    h = (x @ W1 + b1) @ W2 + b2 = x @ Wc + c        (Wc = W1@W2)
    BatchNorm (training mode) is shift-invariant -> the bias c cancels.
    out = LeakyReLU(a * u + b),  u = x @ Wc,
    a = gamma * rsqrt(var+eps),  b = beta - mean * a
    mean = (m @ Wc)/N with m = sum_rows(x);  E[u^2] = diag(Wc^T G Wc)/N with
    G = x^T x;  var = E[u^2] - mean^2.  Sync-BN: one ~66KB AllReduce of
    [G | m] per type.

Schedule (hides both AllReduce+stats latency windows under real work):
    setup Wc -> item-p1 (G,m + SBUF-resident uT store, bf16)
             -> kick AR_item -> user-pA (G,m only; overlaps AR_item)
             -> kick AR_user -> item stats -> item-p2 starts (overlaps AR_user)
             -> user stats   -> item-p2 chunks interleaved with user-pB
                (user recomputes uT from a second read of x_user).

Per row-subtile [128 rows, 128 feats] on the device:
  p1: DMA-cast x f32->bf16 in 1MB chunks (packed: partition p holds qc
      consecutive rows); G += x^T x (PE, accumulating psum); xT = transpose
      (PE, bf16 psum); per 4 subtiles: evict xT (DVE; free-dim accum_out
      gives m for item), uT = Wc^T @ xT (PE, N=512, transposed domain),
      evict uT (ScalarE, cast bf16) -> ut_store [128=d_out, rows].
  p2: LeakyReLU(a*uT+b) as ONE ScalarE Lrelu op (per-partition AP scale/
      bias, alpha); PE transpose back; evict f32 -> staging; DMA out.

x_user [50000,64]/core is viewed host-side as row pairs [25000,128] so both
types share the d=128 path; user Wc is stacked [Wc;Wc], uT uses two K=64
matmuls (even/odd real row of each pair), and the pair-G/pair-m fold their
halves after the AllReduce. Rows are zero-padded host-side to a multiple of
128 (zero rows contribute nothing to G or m).
"""
import sys

for _p in ("/opt/trn_rl_repo",):
    if _p not in sys.path:
        sys.path.insert(0, _p)

import numpy as np

import concourse.mybir as mybir
import concourse.tile as tile
from concourse import bacc
from concourse.masks import make_identity
from concourse.bass_utils import run_bass_kernel_spmd

F32 = mybir.dt.float32
BF16 = mybir.dt.bfloat16
ALU = mybir.AluOpType
AFT = mybir.ActivationFunctionType

NCORES = 8
EPS = 1e-5
NEG_SLOPE = 0.01

N_USER, N_ITEM = 400000, 600000
HID, D_OUT = 256, 128

CHUNK_Q = 16          # packed subtiles per input DMA chunk (1 MB f32)
P2_START_PURE = 8     # item-p2 chunks before user-pB interleave starts
P2_INTERLEAVE = 2     # then 1 user chunk per this many item chunks


def _ceil_to(x, m):
    return (x + m - 1) // m * m


class TypeCfg:
    def __init__(self, name, n_rows_core, d_in, n_total_rows):
        self.name = name
        self.d_in = d_in                        # true d_in: 64 or 128
        self.paired = d_in == 64
        self.rpp = 2 if self.paired else 1      # real rows per packed row
        assert n_rows_core % self.rpp == 0
        self.Np = _ceil_to(n_rows_core // self.rpp, 128)  # padded packed rows
        self.n_rows_core = n_rows_core
        self.n_out_pad = self.Np * self.rpp
        self.N_total = n_total_rows


def _chunks(Np):
    nsub = Np // 128
    out, s = [], 0
    while s < nsub:
        qc = min(CHUNK_Q, nsub - s)
        out.append((s, qc))
        s += qc
    return out


class _Ctx:
    pass


def build_kernel(user, item, ncores=NCORES):
    nc = bacc.Bacc(None, target_bir_lowering=False, num_devices=ncores)
    cfgs = [user, item]

    ext = {}
    for c in cfgs:
        ext[c.name] = {
            "x": nc.declare_dram_parameter(f"x_{c.name}", [c.Np, 128], F32, isOutput=False),
            "W1": nc.declare_dram_parameter(f"W1_{c.name}", [c.d_in, HID], F32, isOutput=False),
            "W2": nc.declare_dram_parameter(f"W2_{c.name}", [HID, D_OUT], F32, isOutput=False),
            "gamma": nc.declare_dram_parameter(f"gamma_{c.name}", [D_OUT], F32, isOutput=False),
            "beta": nc.declare_dram_parameter(f"beta_{c.name}", [D_OUT], F32, isOutput=False),
            "out": nc.declare_dram_parameter(f"out_{c.name}", [c.n_out_pad, D_OUT], F32, isOutput=True),
        }
    ar_in = {c.name: nc.dram_tensor(f"ar_in_{c.name}", [128, 129], F32) for c in cfgs}
    ar_out = {c.name: nc.dram_tensor(f"ar_out_{c.name}", [128, 129], F32, addr_space="Shared")
              for c in cfgs}
    g = _Ctx()
    g.nc, g.rg = nc, [list(range(ncores))]

    with tile.TileContext(nc) as tc:
        g.tc = tc
        with tc.tile_pool(name="const", bufs=1) as constp:
            g.constp = constp
            ident_bf = constp.tile([128, 128], BF16)
            make_identity(nc, ident_bf[:])
            ident_f32 = constp.tile([128, 128], F32)
            make_identity(nc, ident_f32[:])
            ones_f32 = constp.tile([128, 1], F32)
            nc.gpsimd.memset(ones_f32[:], 1.0)
            ones_bf = constp.tile([128, 1], BF16)
            nc.gpsimd.memset(ones_bf[:], 1.0)
            eps_v = constp.tile([128, 1], F32)
            nc.gpsimd.memset(eps_v[:], EPS)
            zero_v = constp.tile([128, 1], F32)
            nc.gpsimd.memset(zero_v[:], 0.0)
            g.ident_bf, g.ident_f32 = ident_bf, ident_f32
            g.ones_f32, g.ones_bf = ones_f32, ones_bf
            g.eps_v, g.zero_v = eps_v, zero_v

            # ---- setup: Wc = W1 @ W2 per type ----
            g.wc_f32, g.wc_stack = {}, {}
            with (
                tc.tile_pool(name="wsetup", bufs=1) as wsp,
                tc.tile_pool(name="wps", bufs=1, space="PSUM") as wps,
            ):
                for c in cfgs:
                    d = c.d_in
                    w1 = wsp.tile([d, HID], F32, tag=f"w1_{c.name}")
                    nc.sync.dma_start(w1[:], ext[c.name]["W1"][:])
                    w2a = wsp.tile([128, D_OUT], F32, tag=f"w2a_{c.name}")
                    w2b = wsp.tile([128, D_OUT], F32, tag=f"w2b_{c.name}")
                    nc.sync.dma_start(w2a[:], ext[c.name]["W2"][0:128, :])
                    nc.sync.dma_start(w2b[:], ext[c.name]["W2"][128:256, :])
                    wc_ps = wps.tile([d, D_OUT], F32, tag="wc")
                    for h, w2h in enumerate((w2a, w2b)):
                        w1t_ps = wps.tile([128, d], F32, tag="w1t")
                        nc.tensor.transpose(w1t_ps[:], w1[:, h * 128:(h + 1) * 128],
                                            ident_f32[0:d, 0:d])
                        w1t = wsp.tile([128, d], F32, tag=f"w1t_{c.name}_{h}")
                        nc.vector.tensor_copy(w1t[:], w1t_ps[:])
                        nc.tensor.matmul(wc_ps[:], w1t[:], w2h[:],
                                         start=(h == 0), stop=(h == 1))
                    wf = constp.tile([d, D_OUT], F32, tag=f"wcf_{c.name}")
                    nc.vector.tensor_copy(wf[:], wc_ps[:])
                    g.wc_f32[c.name] = wf
                    ws = constp.tile([128, D_OUT], BF16, tag=f"wcs_{c.name}")
                    nc.scalar.copy(ws[0:d, :], wc_ps[:])
                    if c.paired:
                        nc.sync.dma_start(ws[64:128, :], ws[0:64, :])
                    g.wc_stack[c.name] = ws

            _phases(g, user, item, ext, ar_in, ar_out)

    nc.finalize()
    return nc


def _phases(g, user, item, ext, ar_in, ar_out):
    nc, tc = g.nc, g.tc
    with (
        tc.tile_pool(name="ut", bufs=1) as utp,
        tc.tile_pool(name="xchp", bufs=3) as xchp,
        tc.tile_pool(name="stgp", bufs=3) as stgp,
        tc.tile_pool(name="sbp", bufs=3) as sbp,
        tc.tile_pool(name="smp", bufs=1) as smp,
        tc.tile_pool(name="psA", bufs=3, space="PSUM") as psA,   # xT / Tback
        tc.tile_pool(name="psB", bufs=2, space="PSUM") as psB,   # uT
        tc.tile_pool(name="psG", bufs=1, space="PSUM") as psG,   # G+m, stats
    ):
        g.utp, g.xchp, g.stgp, g.sbp, g.smp = utp, xchp, stgp, sbp, smp
        g.psA, g.psB, g.psG = psA, psB, psG

        ut_item = utp.tile([128, item.Np], BF16)

        # ---- item p1 (G, m, uT store) ----
        n_groups_i = sum((qc + 3) // 4 for _, qc in _chunks(item.Np))
        m_acc = smp.tile([128, n_groups_i], F32, tag="macc")
        _p1(g, item, ext[item.name], ut_item, m_acc)
        # pack + kick AR_item
        _kick_ar(g, item, ar_in[item.name], m_acc=m_acc, n_acc=n_groups_i)
        nc.gpsimd.collective_compute("AllReduce", ALU.add, replica_groups=g.rg,
                                     ins=[ar_in[item.name][:]],
                                     outs=[ar_out[item.name][:]])

        # ---- user pA (G, m only; overlaps AR_item) ----
        _p1(g, user, ext[user.name], None, None)
        _kick_ar(g, user, ar_in[user.name])
        nc.gpsimd.collective_compute("AllReduce", ALU.add, replica_groups=g.rg,
                                     ins=[ar_in[user.name][:]],
                                     outs=[ar_out[user.name][:]])

        # ---- stats ----
        ab_item = _stats(g, item, ext[item.name], ar_out[item.name])
        ab_user = _stats(g, user, ext[user.name], ar_out[user.name])

        # ---- p2: item chunks interleaved with user-pB chunks ----
        ichunks = _chunks(item.Np)
        uchunks = _chunks(user.Np)
        ui = 0
        for k, (s0, qc) in enumerate(ichunks):
            _p2_item_chunk(g, item, ext[item.name], ut_item, ab_item, s0, qc)
            if (k >= P2_START_PURE and (k - P2_START_PURE) % P2_INTERLEAVE == 0
                    and ui < len(uchunks)):
                us0, uqc = uchunks[ui]
                _pb_user_chunk(g, user, ext[user.name], ab_user, us0, uqc)
                ui += 1
        while ui < len(uchunks):
            us0, uqc = uchunks[ui]
            _pb_user_chunk(g, user, ext[user.name], ab_user, us0, uqc)
            ui += 1


def _p1(g, c, ex, ut_store, m_acc):
    """G (+m) accumulation; if ut_store is not None also transpose+uT+store.
    For the G/m-only variant (user pA), m comes from ones^T x matmuls into
    the G psum bank's columns 128:256."""
    nc = g.nc
    chunks = _chunks(c.Np)
    nsub = c.Np // 128
    full = ut_store is not None
    gm = g.psG.tile([128, 128], F32, tag="gbank")
    g.g_bank = getattr(g, "g_bank", {})
    g.g_bank[c.name] = gm
    g.m_bank = getattr(g, "m_bank", {})
    m_ps = None
    if not full:
        m_ps = g.psG.tile([1, 128], F32, tag="mrow")
    g.m_bank[c.name] = m_ps

    sub_i = 0
    acc_i = 0
    for s0, qc in chunks:
        xch = g.xchp.tile([128, CHUNK_Q * 128], BF16, tag="xch")
        src = ex["x"][s0 * 128:(s0 + qc) * 128, :].rearrange(
            "(p q) d -> p (q d)", p=128)
        nc.gpsimd.dma_start(xch[:, 0:qc * 128], src)   # f32 -> bf16 cast
        for g0 in range(0, qc, 4):
            ns = min(4, qc - g0)
            sls = [xch[:, (g0 + i) * 128:(g0 + i + 1) * 128] for i in range(ns)]
            if full:
                xt_ps = g.psA.tile([128, 512], BF16, tag="xtnat")
            for i in range(ns):
                nc.tensor.matmul(gm[:], sls[i], sls[i],
                                 start=(sub_i == 0), stop=(sub_i == nsub - 1),
                                 skip_group_check=True)
                if full:
                    nc.tensor.transpose(xt_ps[:, i * 128:(i + 1) * 128],
                                        sls[i], g.ident_bf[:])
                else:
                    nc.tensor.matmul(m_ps[:], g.ones_bf[:], sls[i],
                                     start=(sub_i == 0), stop=(sub_i == nsub - 1),
                                     skip_group_check=True)
                sub_i += 1
            if not full:
                continue
            xt_sb = g.sbp.tile([128, 512], BF16, tag="xtsb")
            # evict transposes; accum_out over free dim = m contribution
            nc.vector.tensor_scalar(xt_sb[:, 0:ns * 128], xt_ps[:, 0:ns * 128],
                                    1.0, 0.0, ALU.mult, ALU.add,
                                    accum_out=m_acc[:, acc_i:acc_i + 1])
            acc_i += 1
            blk0 = s0 + g0
            if not c.paired:
                ut_ps = g.psB.tile([128, 512], F32, tag="utps")
                nc.tensor.matmul(ut_ps[:, 0:ns * 128], g.wc_stack[c.name][:],
                                 xt_sb[:, 0:ns * 128], start=True, stop=True)
                nc.scalar.copy(ut_store[:, blk0 * 128:(blk0 + ns) * 128],
                               ut_ps[:, 0:ns * 128])
            else:
                for half in range(2):
                    ut_ps = g.psB.tile([128, 512], F32, tag="utps")
                    nc.tensor.matmul(
                        ut_ps[:, 0:ns * 128],
                        g.wc_stack[c.name][half * 64:(half + 1) * 64, :],
                        xt_sb[half * 64:(half + 1) * 64, 0:ns * 128],
                        start=True, stop=True)
                    dst = ut_store[:, 2 * blk0 * 128:2 * (blk0 + ns) * 128]
                    dst = dst.rearrange("p (n two f) -> p n two f",
                                        two=2, f=128)[:, :, half, :]
                    src3 = ut_ps[:, 0:ns * 128].rearrange("p (n f) -> p n f", f=128)
                    nc.scalar.copy(dst, src3)
    assert sub_i == nsub


def _kick_ar(g, c, ar_in, m_acc=None, n_acc=0):
    nc = g.nc
    gm = g.g_bank[c.name]
    g_sb = g.smp.tile([128, 128], F32, tag=f"gsb_{c.name}")
    nc.vector.tensor_copy(g_sb[:], gm[:])
    nc.sync.dma_start(ar_in[:, 0:128], g_sb[:])
    m_col = g.smp.tile([128, 1], F32, tag=f"mcol_{c.name}")
    if m_acc is not None:
        nc.vector.reduce_sum(m_col[:], m_acc[:, 0:n_acc], axis=mybir.AxisListType.X)
    else:
        # m row [1,128] -> transpose to a per-partition column
        m_row = g.smp.tile([1, 128], F32, tag=f"mrow_{c.name}")
        nc.vector.tensor_copy(m_row[:], g.m_bank[c.name][:])
        mt_ps = g.psG.tile([128, 4], F32, tag="sps")
        nc.tensor.transpose(mt_ps[:, 0:1], m_row[:], g.ident_f32[0:1, 0:1])
        nc.vector.tensor_copy(m_col[:], mt_ps[:, 0:1])
    nc.sync.dma_start(ar_in[:, 128:129], m_col[:])


def _stats(g, c, ex, ar_out):
    """AllReduced [G|m] -> (a_vec, b_vec) [128,1] f32."""
    nc, d = g.nc, c.d_in
    smp, psG = g.smp, g.psG
    name = c.name
    ar_sb = smp.tile([128, 129], F32, tag=f"arsb_{name}")
    nc.sync.dma_start(ar_sb[:], ar_out[:])

    if c.paired:
        tmp = smp.tile([64, 65], F32, tag=f"fold_{name}")
        nc.sync.dma_start(tmp[:], ar_sb[64:128, 64:129])   # cross-partition
        g_eff = smp.tile([64, 64], F32, tag=f"geff_{name}")
        nc.vector.tensor_tensor(g_eff[:], ar_sb[0:64, 0:64], tmp[:, 0:64], ALU.add)
        m_eff = smp.tile([64, 1], F32, tag=f"meff_{name}")
        nc.vector.tensor_tensor(m_eff[:], ar_sb[0:64, 128:129], tmp[:, 64:65],
                                ALU.add)
        g_ap, m_ap = g_eff[:], m_eff[:]
    else:
        g_ap, m_ap = ar_sb[:, 0:128], ar_sb[:, 128:129]

    wc = g.wc_f32[name]
    t1_ps = psG.tile([d, D_OUT], F32, tag="sps")
    nc.tensor.matmul(t1_ps[:], g_ap, wc[:], start=True, stop=True)
    t1 = smp.tile([d, D_OUT], F32, tag=f"t1_{name}")
    nc.vector.tensor_copy(t1[:], t1_ps[:])
    t2 = smp.tile([d, D_OUT], F32, tag=f"t2_{name}")
    nc.vector.tensor_tensor(t2[:], t1[:], wc[:], ALU.mult)
    e2_ps = psG.tile([1, D_OUT], F32, tag="sps")
    nc.tensor.matmul(e2_ps[:], g.ones_f32[0:d, :], t2[:], start=True, stop=True)
    e2_sb = smp.tile([1, D_OUT], F32, tag=f"e2sb_{name}")
    nc.vector.tensor_copy(e2_sb[:], e2_ps[:])
    s_ps = psG.tile([1, D_OUT], F32, tag="sps")
    nc.tensor.matmul(s_ps[:], m_ap, wc[:], start=True, stop=True)
    s_sb = smp.tile([1, D_OUT], F32, tag=f"ssb_{name}")
    nc.vector.tensor_copy(s_sb[:], s_ps[:])
    # assemble 4 rows on partitions 0..3 via DMA (engines cannot cross
    # partitions), then one PE transpose to get per-partition columns
    rowmat = smp.tile([4, 128], F32, tag=f"rowmat_{name}")
    nc.sync.dma_start(rowmat[0:1, :], e2_sb[:])
    nc.sync.dma_start(rowmat[1:2, :], s_sb[:])
    nc.sync.dma_start(rowmat[2:3, :], ex["gamma"][:].rearrange("(o f) -> o f", o=1))
    nc.sync.dma_start(rowmat[3:4, :], ex["beta"][:].rearrange("(o f) -> o f", o=1))
    cols_ps = psG.tile([128, 4], F32, tag="sps")
    nc.tensor.transpose(cols_ps[:], rowmat[:], g.ident_f32[0:4, 0:4])
    cols = smp.tile([128, 4], F32, tag=f"cols_{name}")
    nc.vector.tensor_copy(cols[:], cols_ps[:])

    inv_n = 1.0 / float(c.N_total)
    mean = smp.tile([128, 1], F32, tag=f"mean_{name}")
    nc.vector.tensor_scalar(mean[:], cols[:, 1:2], inv_n, None, ALU.mult)
    msq = smp.tile([128, 1], F32, tag=f"msq_{name}")
    nc.vector.tensor_tensor(msq[:], mean[:], mean[:], ALU.mult)
    e2n = smp.tile([128, 1], F32, tag=f"e2n_{name}")
    nc.vector.tensor_scalar(e2n[:], cols[:, 0:1], inv_n, None, ALU.mult)
    var = smp.tile([128, 1], F32, tag=f"var_{name}")
    nc.vector.tensor_tensor(var[:], e2n[:], msq[:], ALU.subtract)
    lnv = smp.tile([128, 1], F32, tag=f"lnv_{name}")
    nc.scalar.activation(lnv[:], var[:], AFT.Ln, bias=g.eps_v[:], scale=1.0)
    rstd = smp.tile([128, 1], F32, tag=f"rstd_{name}")
    nc.scalar.activation(rstd[:], lnv[:], AFT.Exp, bias=g.zero_v[:], scale=-0.5)
    a_vec = smp.tile([128, 1], F32, tag=f"avec_{name}")
    nc.vector.tensor_tensor(a_vec[:], cols[:, 2:3], rstd[:], ALU.mult)
    ma = smp.tile([128, 1], F32, tag=f"ma_{name}")
    nc.vector.tensor_tensor(ma[:], mean[:], a_vec[:], ALU.mult)
    b_vec = smp.tile([128, 1], F32, tag=f"bvec_{name}")
    nc.vector.tensor_tensor(b_vec[:], cols[:, 3:4], ma[:], ALU.subtract)
    return a_vec, b_vec, {"var": var, "e2n": e2n, "rstd": rstd, "mean": mean,
                          "cols": cols, "arsb": ar_sb, "t2": t2,
                          "geff": g_ap if c.paired else None}


def _p2_item_chunk(g, c, ex, ut_store, ab, s0, qc):
    nc = g.nc
    a_vec, b_vec = ab[0], ab[1]
    for z0 in range(0, qc, 8):
        zn = min(8, qc - z0)
        zw = zn * 128
        lr = g.sbp.tile([128, 1024], BF16, tag="lrelu")
        nc.scalar.activation(lr[:, 0:zw], ut_store[:, (s0 + z0) * 128:
                                                    (s0 + z0) * 128 + zw],
                             AFT.Lrelu, bias=b_vec[:], scale=a_vec[:],
                             alpha=NEG_SLOPE)
        stg = g.stgp.tile([128, 1024], F32, tag="stg")
        for t0 in range(0, zn, 4):
            tn = min(4, zn - t0)
            nat_ps = g.psA.tile([128, 512], BF16, tag="xtnat")
            for i in range(tn):
                nc.tensor.transpose(nat_ps[:, i * 128:(i + 1) * 128],
                                    lr[:, (t0 + i) * 128:(t0 + i + 1) * 128],
                                    g.ident_bf[:])
            nc.any.tensor_copy(stg[:, t0 * 128:(t0 + tn) * 128],
                               nat_ps[:, 0:tn * 128])
        dst = ex["out"][s0 * 128:(s0 + qc) * 128, :].rearrange(
            "(p q) e -> p (q e)", p=128)[:, z0 * 128:z0 * 128 + zw]
        nc.sync.dma_start(dst, stg[:, 0:zw])


def _pb_user_chunk(g, c, ex, ab, s0, qc):
    """user pass B: re-read x, recompute uT, Lrelu from psum, transpose back."""
    nc = g.nc
    a_vec, b_vec = ab[0], ab[1]
    xch = g.xchp.tile([128, CHUNK_Q * 128], BF16, tag="xch")
    src = ex["x"][s0 * 128:(s0 + qc) * 128, :].rearrange("(p q) d -> p (q d)", p=128)
    nc.gpsimd.dma_start(xch[:, 0:qc * 128], src)
    out_rr = ex["out"][s0 * 256:(s0 + qc) * 256, :].rearrange(
        "(p q) e -> p (q e)", p=128)
    for g0 in range(0, qc, 4):
        ns = min(4, qc - g0)
        xt_ps = g.psA.tile([128, 512], BF16, tag="xtnat")
        for i in range(ns):
            nc.tensor.transpose(xt_ps[:, i * 128:(i + 1) * 128],
                                xch[:, (g0 + i) * 128:(g0 + i + 1) * 128],
                                g.ident_bf[:])
        xt_sb = g.sbp.tile([128, 512], BF16, tag="xtsb")
        nc.vector.tensor_copy(xt_sb[:, 0:ns * 128], xt_ps[:, 0:ns * 128])
        stg = g.stgp.tile([128, 1024], F32, tag="stg")
        for half in range(2):
            ut_ps = g.psB.tile([128, 512], F32, tag="utps")
            nc.tensor.matmul(ut_ps[:, 0:ns * 128],
                             g.wc_stack[c.name][half * 64:(half + 1) * 64, :],
                             xt_sb[half * 64:(half + 1) * 64, 0:ns * 128],
                             start=True, stop=True)
            lr = g.sbp.tile([128, 512], BF16, tag="lrelu_u")
            nc.scalar.activation(lr[:, 0:ns * 128], ut_ps[:, 0:ns * 128],
                                 AFT.Lrelu, bias=b_vec[:], scale=a_vec[:],
                                 alpha=NEG_SLOPE)
            nat_ps = g.psA.tile([128, 512], BF16, tag="xtnat")
            for i in range(ns):
                nc.tensor.transpose(nat_ps[:, i * 128:(i + 1) * 128],
                                    lr[:, i * 128:(i + 1) * 128], g.ident_bf[:])
            # real blocks 2i+half within this group -> strided staging cols
            dst = stg[:, 0:2 * ns * 128].rearrange(
                "p (n two f) -> p n two f", two=2, f=128)[:, :, half, :]
            nc.any.tensor_copy(dst, nat_ps[:, 0:ns * 128].rearrange(
                "p (n f) -> p n f", f=128))
        dcols = 2 * ns * 128
        nc.sync.dma_start(out_rr[:, 2 * g0 * 128:2 * g0 * 128 + dcols],
                          stg[:, 0:dcols])


# ---------------------------------------------------------------------------
_BUILT = {}


def _get_built():
    if "full" not in _BUILT:
        user = TypeCfg("user", N_USER // NCORES, 64, N_USER)
        item = TypeCfg("item", N_ITEM // NCORES, 128, N_ITEM)
        _BUILT["full"] = (build_kernel(user, item), user, item)
    return _BUILT["full"]


def kernel(x_user, x_item,
           W1_user=None, b1_user=None, W1_item=None, b1_item=None,
           W2_user=None, b2_user=None, W2_item=None, b2_item=None,
           gamma_user=None, beta_user=None, gamma_item=None, beta_item=None,
           _trace=False):
    nc, ucfg, icfg = _get_built()

    def prep(x, cfg):
        x = np.ascontiguousarray(np.asarray(x, np.float32))
        n = x.shape[0] // NCORES
        shards = []
        for i in range(NCORES):
            s = x[i * n:(i + 1) * n].reshape(-1, 128)
            pad = cfg.Np - s.shape[0]
            if pad:
                s = np.concatenate([s, np.zeros((pad, 128), np.float32)], 0)
            shards.append(s)
        return shards

    xu = prep(x_user, ucfg)
    xi = prep(x_item, icfg)
    common = {
        "W1_user": np.asarray(W1_user, np.float32),
        "W2_user": np.asarray(W2_user, np.float32),
        "gamma_user": np.asarray(gamma_user, np.float32),
        "beta_user": np.asarray(beta_user, np.float32),
        "W1_item": np.asarray(W1_item, np.float32),
        "W2_item": np.asarray(W2_item, np.float32),
        "gamma_item": np.asarray(gamma_item, np.float32),
        "beta_item": np.asarray(beta_item, np.float32),
    }
    in_maps = [dict(common, x_user=xu[i], x_item=xi[i]) for i in range(NCORES)]
    res = run_bass_kernel_spmd(nc, in_maps, list(range(NCORES)), trace=_trace)
    nu, ni = N_USER // NCORES, N_ITEM // NCORES
    out_user = np.concatenate(
        [res.results[i]["out_user"][:nu] for i in range(NCORES)], 0)
    out_item = np.concatenate(
        [res.results[i]["out_item"][:ni] for i in range(NCORES)], 0)
    if _trace:
        kernel.last_exec_time_ns = res.exec_time_ns
    return (out_user, out_item)


# revision 19
# speedup vs baseline: 1.4866x; 1.0325x over previous
"""Trainium2 Bass kernel for nn_ActivatedHeteroLinear (moe_routing, 8 cores).

Math: per type t in {user, item}:
    h = (x @ W1 + b1) @ W2 + b2 = x @ Wc + c        (Wc = W1@W2)
    BatchNorm (training mode) is shift-invariant -> the bias c cancels.
    out = LeakyReLU(a * u + b),  u = x @ Wc,
    a = gamma * rsqrt(var+eps),  b = beta - mean * a
    mean = (m @ Wc)/N with m = sum_rows(x);  E[u^2] = diag(Wc^T G Wc)/N with
    G = x^T x;  var = E[u^2] - mean^2.  Sync-BN: one ~66KB AllReduce of
    [G | m] per type.

Schedule (hides both AllReduce+stats latency windows under real work):
    setup Wc -> item-p1 (G,m + SBUF-resident uT store, bf16)
             -> kick AR_item -> user-pA (G,m only; overlaps AR_item)
             -> kick AR_user -> item stats -> item-p2 starts (overlaps AR_user)
             -> user stats   -> item-p2 chunks interleaved with user-pB
                (user recomputes uT from a second read of x_user).

Per row-subtile [128 rows, 128 feats] on the device:
  p1: DMA-cast x f32->bf16 in 1MB chunks (packed: partition p holds qc
      consecutive rows); G += x^T x (PE, accumulating psum); xT = transpose
      (PE, bf16 psum); per 4 subtiles: evict xT (DVE; free-dim accum_out
      gives m for item), uT = Wc^T @ xT (PE, N=512, transposed domain),
      evict uT (ScalarE, cast bf16) -> ut_store [128=d_out, rows].
  p2: LeakyReLU(a*uT+b) as ONE ScalarE Lrelu op (per-partition AP scale/
      bias, alpha); PE transpose back; evict f32 -> staging; DMA out.

x_user [50000,64]/core is viewed host-side as row pairs [25000,128] so both
types share the d=128 path; user Wc is stacked [Wc;Wc], uT uses two K=64
matmuls (even/odd real row of each pair), and the pair-G/pair-m fold their
halves after the AllReduce. Rows are zero-padded host-side to a multiple of
128 (zero rows contribute nothing to G or m).
"""
import sys

for _p in ("/opt/trn_rl_repo",):
    if _p not in sys.path:
        sys.path.insert(0, _p)

import numpy as np

import concourse.mybir as mybir
import concourse.tile as tile
from concourse import bacc
from concourse.masks import make_identity
from concourse.bass_utils import run_bass_kernel_spmd

F32 = mybir.dt.float32
BF16 = mybir.dt.bfloat16
ALU = mybir.AluOpType
AFT = mybir.ActivationFunctionType

NCORES = 8
EPS = 1e-5
NEG_SLOPE = 0.01

N_USER, N_ITEM = 400000, 600000
HID, D_OUT = 256, 128

CHUNK_Q = 16          # packed subtiles per input DMA chunk (1 MB f32)
P2_START_PURE = 8     # item-p2 chunks before user-pB interleave starts
P2_INTERLEAVE = 2     # then 1 user chunk per this many item chunks


def _ceil_to(x, m):
    return (x + m - 1) // m * m


class TypeCfg:
    def __init__(self, name, n_rows_core, d_in, n_total_rows):
        self.name = name
        self.d_in = d_in                        # true d_in: 64 or 128
        self.paired = d_in == 64
        self.rpp = 2 if self.paired else 1      # real rows per packed row
        assert n_rows_core % self.rpp == 0
        self.Np = _ceil_to(n_rows_core // self.rpp, 128)  # padded packed rows
        self.n_rows_core = n_rows_core
        self.n_out_pad = self.Np * self.rpp
        self.N_total = n_total_rows


def _chunks(Np):
    nsub = Np // 128
    out, s = [], 0
    while s < nsub:
        qc = min(CHUNK_Q, nsub - s)
        out.append((s, qc))
        s += qc
    return out


class _Ctx:
    pass


def build_kernel(user, item, ncores=NCORES):
    nc = bacc.Bacc(None, target_bir_lowering=False, num_devices=ncores)
    cfgs = [user, item]

    ext = {}
    for c in cfgs:
        ext[c.name] = {
            "x": nc.declare_dram_parameter(f"x_{c.name}", [c.Np, 128], F32, isOutput=False),
            "W1": nc.declare_dram_parameter(f"W1_{c.name}", [c.d_in, HID], F32, isOutput=False),
            "W2": nc.declare_dram_parameter(f"W2_{c.name}", [HID, D_OUT], F32, isOutput=False),
            "gamma": nc.declare_dram_parameter(f"gamma_{c.name}", [D_OUT], F32, isOutput=False),
            "beta": nc.declare_dram_parameter(f"beta_{c.name}", [D_OUT], F32, isOutput=False),
            "out": nc.declare_dram_parameter(f"out_{c.name}", [c.n_out_pad, D_OUT], F32, isOutput=True),
        }
    ar_in = {c.name: nc.dram_tensor(f"ar_in_{c.name}", [128, 129], F32) for c in cfgs}
    ar_out = {c.name: nc.dram_tensor(f"ar_out_{c.name}", [128, 129], F32, addr_space="Shared")
              for c in cfgs}
    g = _Ctx()
    g.nc, g.rg = nc, [list(range(ncores))]

    with tile.TileContext(nc) as tc:
        g.tc = tc
        with tc.tile_pool(name="const", bufs=1) as constp:
            g.constp = constp
            ident_bf = constp.tile([128, 128], BF16)
            make_identity(nc, ident_bf[:])
            ident_f32 = constp.tile([128, 128], F32)
            make_identity(nc, ident_f32[:])
            ones_f32 = constp.tile([128, 1], F32)
            nc.gpsimd.memset(ones_f32[:], 1.0)
            ones_bf = constp.tile([128, 1], BF16)
            nc.gpsimd.memset(ones_bf[:], 1.0)
            eps_v = constp.tile([128, 1], F32)
            nc.gpsimd.memset(eps_v[:], EPS)
            zero_v = constp.tile([128, 1], F32)
            nc.gpsimd.memset(zero_v[:], 0.0)
            g.ident_bf, g.ident_f32 = ident_bf, ident_f32
            g.ones_f32, g.ones_bf = ones_f32, ones_bf
            g.eps_v, g.zero_v = eps_v, zero_v

            # ---- setup: Wc = W1 @ W2 per type ----
            g.wc_f32, g.wc_stack = {}, {}
            with (
                tc.tile_pool(name="wsetup", bufs=1) as wsp,
                tc.tile_pool(name="wps", bufs=1, space="PSUM") as wps,
            ):
                for c in cfgs:
                    d = c.d_in
                    w1 = wsp.tile([d, HID], F32, tag=f"w1_{c.name}")
                    nc.sync.dma_start(w1[:], ext[c.name]["W1"][:])
                    w2a = wsp.tile([128, D_OUT], F32, tag=f"w2a_{c.name}")
                    w2b = wsp.tile([128, D_OUT], F32, tag=f"w2b_{c.name}")
                    nc.sync.dma_start(w2a[:], ext[c.name]["W2"][0:128, :])
                    nc.sync.dma_start(w2b[:], ext[c.name]["W2"][128:256, :])
                    wc_ps = wps.tile([d, D_OUT], F32, tag="wc")
                    for h, w2h in enumerate((w2a, w2b)):
                        w1t_ps = wps.tile([128, d], F32, tag="w1t")
                        nc.tensor.transpose(w1t_ps[:], w1[:, h * 128:(h + 1) * 128],
                                            ident_f32[0:d, 0:d])
                        w1t = wsp.tile([128, d], F32, tag=f"w1t_{c.name}_{h}")
                        nc.vector.tensor_copy(w1t[:], w1t_ps[:])
                        nc.tensor.matmul(wc_ps[:], w1t[:], w2h[:],
                                         start=(h == 0), stop=(h == 1))
                    wf = constp.tile([d, D_OUT], F32, tag=f"wcf_{c.name}")
                    nc.vector.tensor_copy(wf[:], wc_ps[:])
                    g.wc_f32[c.name] = wf
                    ws = constp.tile([128, D_OUT], BF16, tag=f"wcs_{c.name}")
                    nc.scalar.copy(ws[0:d, :], wc_ps[:])
                    if c.paired:
                        nc.sync.dma_start(ws[64:128, :], ws[0:64, :])
                    g.wc_stack[c.name] = ws

            _phases(g, user, item, ext, ar_in, ar_out)

    nc.finalize()
    return nc


def _phases(g, user, item, ext, ar_in, ar_out):
    nc, tc = g.nc, g.tc
    with (
        tc.tile_pool(name="ut", bufs=1) as utp,
        tc.tile_pool(name="xchp", bufs=3) as xchp,
        tc.tile_pool(name="stgp", bufs=3) as stgp,
        tc.tile_pool(name="sbp", bufs=3) as sbp,
        tc.tile_pool(name="smp", bufs=1) as smp,
        tc.tile_pool(name="psA", bufs=3, space="PSUM") as psA,   # xT / Tback
        tc.tile_pool(name="psB", bufs=2, space="PSUM") as psB,   # uT
        tc.tile_pool(name="psG", bufs=1, space="PSUM") as psG,   # G+m, stats
    ):
        g.utp, g.xchp, g.stgp, g.sbp, g.smp = utp, xchp, stgp, sbp, smp
        g.psA, g.psB, g.psG = psA, psB, psG

        ut_item = utp.tile([128, item.Np], BF16)

        # ---- item p1 (G, m, uT store) ----
        n_groups_i = sum((qc + 3) // 4 for _, qc in _chunks(item.Np))
        m_acc = smp.tile([128, n_groups_i], F32, tag="macc")
        _p1(g, item, ext[item.name], ut_item, m_acc)
        # pack + kick AR_item
        _kick_ar(g, item, ar_in[item.name], m_acc=m_acc, n_acc=n_groups_i)
        nc.gpsimd.collective_compute("AllReduce", ALU.add, replica_groups=g.rg,
                                     ins=[ar_in[item.name][:]],
                                     outs=[ar_out[item.name][:]])

        # ---- user pA (G, m only; overlaps AR_item) ----
        _p1(g, user, ext[user.name], None, None)
        _kick_ar(g, user, ar_in[user.name])
        nc.gpsimd.collective_compute("AllReduce", ALU.add, replica_groups=g.rg,
                                     ins=[ar_in[user.name][:]],
                                     outs=[ar_out[user.name][:]])

        # ---- stats ----
        ab_item = _stats(g, item, ext[item.name], ar_out[item.name])
        ab_user = _stats(g, user, ext[user.name], ar_out[user.name])

        # ---- p2: item chunks interleaved with user-pB chunks ----
        ichunks = _chunks(item.Np)
        uchunks = _chunks(user.Np)
        ui = 0
        for k, (s0, qc) in enumerate(ichunks):
            _p2_item_chunk(g, item, ext[item.name], ut_item, ab_item, s0, qc)
            if (k >= P2_START_PURE and (k - P2_START_PURE) % P2_INTERLEAVE == 0
                    and ui < len(uchunks)):
                us0, uqc = uchunks[ui]
                _pb_user_chunk(g, user, ext[user.name], ab_user, us0, uqc)
                ui += 1
        while ui < len(uchunks):
            us0, uqc = uchunks[ui]
            _pb_user_chunk(g, user, ext[user.name], ab_user, us0, uqc)
            ui += 1


def _p1(g, c, ex, ut_store, m_acc):
    """G (+m) accumulation; if ut_store is not None also transpose+uT+store.
    For the G/m-only variant (user pA), m comes from ones^T x matmuls into
    the G psum bank's columns 128:256."""
    nc = g.nc
    chunks = _chunks(c.Np)
    nsub = c.Np // 128
    full = ut_store is not None
    gm = g.psG.tile([128, 128], F32, tag="gbank")
    g.g_bank = getattr(g, "g_bank", {})
    g.g_bank[c.name] = gm
    g.m_bank = getattr(g, "m_bank", {})
    m_ps = None
    if not full:
        m_ps = g.psG.tile([1, 128], F32, tag="mrow")
    g.m_bank[c.name] = m_ps

    def flush(pend):
        """downstream of one group: evict xT, uT matmul(s), store uT."""
        xt_ps, ns, blk0, slot = pend
        xt_sb = g.sbp.tile([128, 512], BF16, tag="xtsb")
        nc.vector.tensor_scalar(xt_sb[:, 0:ns * 128], xt_ps[:, 0:ns * 128],
                                1.0, 0.0, ALU.mult, ALU.add,
                                accum_out=m_acc[:, slot:slot + 1])
        if not c.paired:
            ut_ps = g.psB.tile([128, 512], F32, tag="utps")
            nc.tensor.matmul(ut_ps[:, 0:ns * 128], g.wc_stack[c.name][:],
                             xt_sb[:, 0:ns * 128], start=True, stop=True)
            nc.scalar.copy(ut_store[:, blk0 * 128:(blk0 + ns) * 128],
                           ut_ps[:, 0:ns * 128])
        else:
            for half in range(2):
                ut_ps = g.psB.tile([128, 512], F32, tag="utps")
                nc.tensor.matmul(
                    ut_ps[:, 0:ns * 128],
                    g.wc_stack[c.name][half * 64:(half + 1) * 64, :],
                    xt_sb[half * 64:(half + 1) * 64, 0:ns * 128],
                    start=True, stop=True)
                dst = ut_store[:, 2 * blk0 * 128:2 * (blk0 + ns) * 128]
                dst = dst.rearrange("p (n two f) -> p n two f",
                                    two=2, f=128)[:, :, half, :]
                src3 = ut_ps[:, 0:ns * 128].rearrange("p (n f) -> p n f", f=128)
                nc.scalar.copy(dst, src3)

    sub_i = 0
    acc_i = 0
    pending = None   # one-group software-pipeline skew keeps PE fed
    for s0, qc in chunks:
        xch = g.xchp.tile([128, CHUNK_Q * 128], BF16, tag="xch")
        src = ex["x"][s0 * 128:(s0 + qc) * 128, :].rearrange(
            "(p q) d -> p (q d)", p=128)
        nc.gpsimd.dma_start(xch[:, 0:qc * 128], src)   # f32 -> bf16 cast
        for g0 in range(0, qc, 4):
            ns = min(4, qc - g0)
            sls = [xch[:, (g0 + i) * 128:(g0 + i + 1) * 128] for i in range(ns)]
            if full:
                xt_ps = g.psA.tile([128, 512], BF16, tag="xtnat")
            else:
                xt_ps = None
            for i in range(ns):
                nc.tensor.matmul(gm[:], sls[i], sls[i],
                                 start=(sub_i == 0), stop=(sub_i == nsub - 1),
                                 skip_group_check=True)
                if full:
                    nc.tensor.transpose(xt_ps[:, i * 128:(i + 1) * 128],
                                        sls[i], g.ident_bf[:])
                else:
                    nc.tensor.matmul(m_ps[:], g.ones_bf[:], sls[i],
                                     start=(sub_i == 0), stop=(sub_i == nsub - 1),
                                     skip_group_check=True)
                sub_i += 1
            if not full:
                continue
            if pending is not None:
                flush(pending)
            pending = (xt_ps, ns, s0 + g0, acc_i)
            acc_i += 1
    if pending is not None:
        flush(pending)
    assert sub_i == nsub


def _kick_ar(g, c, ar_in, m_acc=None, n_acc=0):
    nc = g.nc
    gm = g.g_bank[c.name]
    g_sb = g.smp.tile([128, 128], F32, tag=f"gsb_{c.name}")
    nc.vector.tensor_copy(g_sb[:], gm[:])
    nc.sync.dma_start(ar_in[:, 0:128], g_sb[:])
    m_col = g.smp.tile([128, 1], F32, tag=f"mcol_{c.name}")
    if m_acc is not None:
        nc.vector.reduce_sum(m_col[:], m_acc[:, 0:n_acc], axis=mybir.AxisListType.X)
    else:
        # m row [1,128] -> transpose to a per-partition column
        m_row = g.smp.tile([1, 128], F32, tag=f"mrow_{c.name}")
        nc.vector.tensor_copy(m_row[:], g.m_bank[c.name][:])
        mt_ps = g.psG.tile([128, 4], F32, tag="sps")
        nc.tensor.transpose(mt_ps[:, 0:1], m_row[:], g.ident_f32[0:1, 0:1])
        nc.vector.tensor_copy(m_col[:], mt_ps[:, 0:1])
    nc.sync.dma_start(ar_in[:, 128:129], m_col[:])


def _stats(g, c, ex, ar_out):
    """AllReduced [G|m] -> (a_vec, b_vec) [128,1] f32."""
    nc, d = g.nc, c.d_in
    smp, psG = g.smp, g.psG
    name = c.name
    ar_sb = smp.tile([128, 129], F32, tag=f"arsb_{name}")
    nc.sync.dma_start(ar_sb[:], ar_out[:])

    if c.paired:
        tmp = smp.tile([64, 65], F32, tag=f"fold_{name}")
        nc.sync.dma_start(tmp[:], ar_sb[64:128, 64:129])   # cross-partition
        g_eff = smp.tile([64, 64], F32, tag=f"geff_{name}")
        nc.vector.tensor_tensor(g_eff[:], ar_sb[0:64, 0:64], tmp[:, 0:64], ALU.add)
        m_eff = smp.tile([64, 1], F32, tag=f"meff_{name}")
        nc.vector.tensor_tensor(m_eff[:], ar_sb[0:64, 128:129], tmp[:, 64:65],
                                ALU.add)
        g_ap, m_ap = g_eff[:], m_eff[:]
    else:
        g_ap, m_ap = ar_sb[:, 0:128], ar_sb[:, 128:129]

    wc = g.wc_f32[name]
    t1_ps = psG.tile([d, D_OUT], F32, tag="sps")
    nc.tensor.matmul(t1_ps[:], g_ap, wc[:], start=True, stop=True)
    t1 = smp.tile([d, D_OUT], F32, tag=f"t1_{name}")
    nc.vector.tensor_copy(t1[:], t1_ps[:])
    t2 = smp.tile([d, D_OUT], F32, tag=f"t2_{name}")
    nc.vector.tensor_tensor(t2[:], t1[:], wc[:], ALU.mult)
    e2_ps = psG.tile([1, D_OUT], F32, tag="sps")
    nc.tensor.matmul(e2_ps[:], g.ones_f32[0:d, :], t2[:], start=True, stop=True)
    e2_sb = smp.tile([1, D_OUT], F32, tag=f"e2sb_{name}")
    nc.vector.tensor_copy(e2_sb[:], e2_ps[:])
    s_ps = psG.tile([1, D_OUT], F32, tag="sps")
    nc.tensor.matmul(s_ps[:], m_ap, wc[:], start=True, stop=True)
    s_sb = smp.tile([1, D_OUT], F32, tag=f"ssb_{name}")
    nc.vector.tensor_copy(s_sb[:], s_ps[:])
    # assemble 4 rows on partitions 0..3 via DMA (engines cannot cross
    # partitions), then one PE transpose to get per-partition columns
    rowmat = smp.tile([4, 128], F32, tag=f"rowmat_{name}")
    nc.sync.dma_start(rowmat[0:1, :], e2_sb[:])
    nc.sync.dma_start(rowmat[1:2, :], s_sb[:])
    nc.sync.dma_start(rowmat[2:3, :], ex["gamma"][:].rearrange("(o f) -> o f", o=1))
    nc.sync.dma_start(rowmat[3:4, :], ex["beta"][:].rearrange("(o f) -> o f", o=1))
    cols_ps = psG.tile([128, 4], F32, tag="sps")
    nc.tensor.transpose(cols_ps[:], rowmat[:], g.ident_f32[0:4, 0:4])
    cols = smp.tile([128, 4], F32, tag=f"cols_{name}")
    nc.vector.tensor_copy(cols[:], cols_ps[:])

    inv_n = 1.0 / float(c.N_total)
    mean = smp.tile([128, 1], F32, tag=f"mean_{name}")
    nc.vector.tensor_scalar(mean[:], cols[:, 1:2], inv_n, None, ALU.mult)
    msq = smp.tile([128, 1], F32, tag=f"msq_{name}")
    nc.vector.tensor_tensor(msq[:], mean[:], mean[:], ALU.mult)
    e2n = smp.tile([128, 1], F32, tag=f"e2n_{name}")
    nc.vector.tensor_scalar(e2n[:], cols[:, 0:1], inv_n, None, ALU.mult)
    var = smp.tile([128, 1], F32, tag=f"var_{name}")
    nc.vector.tensor_tensor(var[:], e2n[:], msq[:], ALU.subtract)
    lnv = smp.tile([128, 1], F32, tag=f"lnv_{name}")
    nc.scalar.activation(lnv[:], var[:], AFT.Ln, bias=g.eps_v[:], scale=1.0)
    rstd = smp.tile([128, 1], F32, tag=f"rstd_{name}")
    nc.scalar.activation(rstd[:], lnv[:], AFT.Exp, bias=g.zero_v[:], scale=-0.5)
    a_vec = smp.tile([128, 1], F32, tag=f"avec_{name}")
    nc.vector.tensor_tensor(a_vec[:], cols[:, 2:3], rstd[:], ALU.mult)
    ma = smp.tile([128, 1], F32, tag=f"ma_{name}")
    nc.vector.tensor_tensor(ma[:], mean[:], a_vec[:], ALU.mult)
    b_vec = smp.tile([128, 1], F32, tag=f"bvec_{name}")
    nc.vector.tensor_tensor(b_vec[:], cols[:, 3:4], ma[:], ALU.subtract)
    return a_vec, b_vec, {"var": var, "e2n": e2n, "rstd": rstd, "mean": mean,
                          "cols": cols, "arsb": ar_sb, "t2": t2,
                          "geff": g_ap if c.paired else None}


def _p2_item_chunk(g, c, ex, ut_store, ab, s0, qc):
    nc = g.nc
    a_vec, b_vec = ab[0], ab[1]
    zgs = []
    # emit the whole chunk's Lrelu ops first so ScalarE runs ahead of PE
    for z0 in range(0, qc, 8):
        zn = min(8, qc - z0)
        zw = zn * 128
        lr = g.sbp.tile([128, 1024], BF16, tag="lrelu")
        nc.scalar.activation(lr[:, 0:zw], ut_store[:, (s0 + z0) * 128:
                                                    (s0 + z0) * 128 + zw],
                             AFT.Lrelu, bias=b_vec[:], scale=a_vec[:],
                             alpha=NEG_SLOPE)
        zgs.append((z0, zn, zw, lr))
    for z0, zn, zw, lr in zgs:
        stg = g.stgp.tile([128, 1024], F32, tag="stg")
        for t0 in range(0, zn, 4):
            tn = min(4, zn - t0)
            nat_ps = g.psA.tile([128, 512], BF16, tag="xtnat")
            for i in range(tn):
                nc.tensor.transpose(nat_ps[:, i * 128:(i + 1) * 128],
                                    lr[:, (t0 + i) * 128:(t0 + i + 1) * 128],
                                    g.ident_bf[:])
            nc.any.tensor_copy(stg[:, t0 * 128:(t0 + tn) * 128],
                               nat_ps[:, 0:tn * 128])
        dst = ex["out"][s0 * 128:(s0 + qc) * 128, :].rearrange(
            "(p q) e -> p (q e)", p=128)[:, z0 * 128:z0 * 128 + zw]
        nc.sync.dma_start(dst, stg[:, 0:zw])


def _pb_user_chunk(g, c, ex, ab, s0, qc):
    """user pass B: re-read x, recompute uT, Lrelu from psum, transpose back."""
    nc = g.nc
    a_vec, b_vec = ab[0], ab[1]
    xch = g.xchp.tile([128, CHUNK_Q * 128], BF16, tag="xch")
    src = ex["x"][s0 * 128:(s0 + qc) * 128, :].rearrange("(p q) d -> p (q d)", p=128)
    nc.gpsimd.dma_start(xch[:, 0:qc * 128], src)
    out_rr = ex["out"][s0 * 256:(s0 + qc) * 256, :].rearrange(
        "(p q) e -> p (q e)", p=128)

    def flush(pend):
        g0, ns, xt_ps = pend
        xt_sb = g.sbp.tile([128, 512], BF16, tag="xtsb")
        nc.vector.tensor_copy(xt_sb[:, 0:ns * 128], xt_ps[:, 0:ns * 128])
        stg = g.stgp.tile([128, 1024], F32, tag="stg")
        for half in range(2):
            ut_ps = g.psB.tile([128, 512], F32, tag="utps")
            nc.tensor.matmul(ut_ps[:, 0:ns * 128],
                             g.wc_stack[c.name][half * 64:(half + 1) * 64, :],
                             xt_sb[half * 64:(half + 1) * 64, 0:ns * 128],
                             start=True, stop=True)
            lr = g.sbp.tile([128, 512], BF16, tag="lrelu_u")
            nc.scalar.activation(lr[:, 0:ns * 128], ut_ps[:, 0:ns * 128],
                                 AFT.Lrelu, bias=b_vec[:], scale=a_vec[:],
                                 alpha=NEG_SLOPE)
            nat_ps = g.psA.tile([128, 512], BF16, tag="xtnat")
            for i in range(ns):
                nc.tensor.transpose(nat_ps[:, i * 128:(i + 1) * 128],
                                    lr[:, i * 128:(i + 1) * 128], g.ident_bf[:])
            # real blocks 2i+half within this group -> strided staging cols
            dst = stg[:, 0:2 * ns * 128].rearrange(
                "p (n two f) -> p n two f", two=2, f=128)[:, :, half, :]
            nc.any.tensor_copy(dst, nat_ps[:, 0:ns * 128].rearrange(
                "p (n f) -> p n f", f=128))
        dcols = 2 * ns * 128
        nc.sync.dma_start(out_rr[:, 2 * g0 * 128:2 * g0 * 128 + dcols],
                          stg[:, 0:dcols])

    pending = None
    for g0 in range(0, qc, 4):
        ns = min(4, qc - g0)
        xt_ps = g.psA.tile([128, 512], BF16, tag="xtnat")
        for i in range(ns):
            nc.tensor.transpose(xt_ps[:, i * 128:(i + 1) * 128],
                                xch[:, (g0 + i) * 128:(g0 + i + 1) * 128],
                                g.ident_bf[:])
        if pending is not None:
            flush(pending)
        pending = (g0, ns, xt_ps)
    if pending is not None:
        flush(pending)


# ---------------------------------------------------------------------------
_BUILT = {}


def _get_built():
    if "full" not in _BUILT:
        user = TypeCfg("user", N_USER // NCORES, 64, N_USER)
        item = TypeCfg("item", N_ITEM // NCORES, 128, N_ITEM)
        _BUILT["full"] = (build_kernel(user, item), user, item)
    return _BUILT["full"]


def kernel(x_user, x_item,
           W1_user=None, b1_user=None, W1_item=None, b1_item=None,
           W2_user=None, b2_user=None, W2_item=None, b2_item=None,
           gamma_user=None, beta_user=None, gamma_item=None, beta_item=None,
           _trace=False):
    nc, ucfg, icfg = _get_built()

    def prep(x, cfg):
        x = np.ascontiguousarray(np.asarray(x, np.float32))
        n = x.shape[0] // NCORES
        shards = []
        for i in range(NCORES):
            s = x[i * n:(i + 1) * n].reshape(-1, 128)
            pad = cfg.Np - s.shape[0]
            if pad:
                s = np.concatenate([s, np.zeros((pad, 128), np.float32)], 0)
            shards.append(s)
        return shards

    xu = prep(x_user, ucfg)
    xi = prep(x_item, icfg)
    common = {
        "W1_user": np.asarray(W1_user, np.float32),
        "W2_user": np.asarray(W2_user, np.float32),
        "gamma_user": np.asarray(gamma_user, np.float32),
        "beta_user": np.asarray(beta_user, np.float32),
        "W1_item": np.asarray(W1_item, np.float32),
        "W2_item": np.asarray(W2_item, np.float32),
        "gamma_item": np.asarray(gamma_item, np.float32),
        "beta_item": np.asarray(beta_item, np.float32),
    }
    in_maps = [dict(common, x_user=xu[i], x_item=xi[i]) for i in range(NCORES)]
    res = run_bass_kernel_spmd(nc, in_maps, list(range(NCORES)), trace=_trace)
    nu, ni = N_USER // NCORES, N_ITEM // NCORES
    out_user = np.concatenate(
        [res.results[i]["out_user"][:nu] for i in range(NCORES)], 0)
    out_item = np.concatenate(
        [res.results[i]["out_item"][:ni] for i in range(NCORES)], 0)
    if _trace:
        kernel.last_exec_time_ns = res.exec_time_ns
    return (out_user, out_item)


# revision 21
# speedup vs baseline: 1.5041x; 1.0117x over previous
"""Trainium2 Bass kernel for nn_ActivatedHeteroLinear (moe_routing, 8 cores).

Math: per type t in {user, item}:
    h = (x @ W1 + b1) @ W2 + b2 = x @ Wc + c        (Wc = W1@W2)
    BatchNorm (training mode) is shift-invariant -> the bias c cancels.
    out = LeakyReLU(a * u + b),  u = x @ Wc,
    a = gamma * rsqrt(var+eps),  b = beta - mean * a
    mean = (m @ Wc)/N with m = sum_rows(x);  E[u^2] = diag(Wc^T G Wc)/N with
    G = x^T x;  var = E[u^2] - mean^2.  Sync-BN: one ~66KB AllReduce of
    [G | m] per type.

Schedule (hides both AllReduce+stats latency windows under real work):
    setup Wc -> item-p1 (G,m + SBUF-resident uT store, bf16)
             -> kick AR_item -> user-pA (G,m only; overlaps AR_item)
             -> kick AR_user -> item stats -> item-p2 starts (overlaps AR_user)
             -> user stats   -> item-p2 chunks interleaved with user-pB
                (user recomputes uT from a second read of x_user).

Per row-subtile [128 rows, 128 feats] on the device:
  p1: DMA-cast x f32->bf16 in 1MB chunks (packed: partition p holds qc
      consecutive rows); G += x^T x (PE, accumulating psum); xT = transpose
      (PE, bf16 psum); per 4 subtiles: evict xT (DVE; free-dim accum_out
      gives m for item), uT = Wc^T @ xT (PE, N=512, transposed domain),
      evict uT (ScalarE, cast bf16) -> ut_store [128=d_out, rows].
  p2: LeakyReLU(a*uT+b) as ONE ScalarE Lrelu op (per-partition AP scale/
      bias, alpha); PE transpose back; evict f32 -> staging; DMA out.

x_user [50000,64]/core is viewed host-side as row pairs [25000,128] so both
types share the d=128 path; user Wc is stacked [Wc;Wc], uT uses two K=64
matmuls (even/odd real row of each pair), and the pair-G/pair-m fold their
halves after the AllReduce. Rows are zero-padded host-side to a multiple of
128 (zero rows contribute nothing to G or m).
"""
import sys

for _p in ("/opt/trn_rl_repo",):
    if _p not in sys.path:
        sys.path.insert(0, _p)

import numpy as np

import concourse.mybir as mybir
import concourse.tile as tile
from concourse import bacc
from concourse.masks import make_identity
from concourse.bass_utils import run_bass_kernel_spmd

F32 = mybir.dt.float32
BF16 = mybir.dt.bfloat16
ALU = mybir.AluOpType
AFT = mybir.ActivationFunctionType

NCORES = 8
EPS = 1e-5
NEG_SLOPE = 0.01

N_USER, N_ITEM = 400000, 600000
HID, D_OUT = 256, 128

CHUNK_Q = 16          # packed subtiles per input DMA chunk (1 MB f32)
P2_START_PURE = 8     # item-p2 chunks before user-pB interleave starts
P2_INTERLEAVE = 2     # then 1 user chunk per this many item chunks


def _ceil_to(x, m):
    return (x + m - 1) // m * m


class TypeCfg:
    def __init__(self, name, n_rows_core, d_in, n_total_rows):
        self.name = name
        self.d_in = d_in                        # true d_in: 64 or 128
        self.paired = d_in == 64
        self.rpp = 2 if self.paired else 1      # real rows per packed row
        assert n_rows_core % self.rpp == 0
        self.Np = _ceil_to(n_rows_core // self.rpp, 128)  # padded packed rows
        self.n_rows_core = n_rows_core
        self.n_out_pad = self.Np * self.rpp
        self.N_total = n_total_rows


def _chunks(Np):
    nsub = Np // 128
    out, s = [], 0
    while s < nsub:
        qc = min(CHUNK_Q, nsub - s)
        out.append((s, qc))
        s += qc
    return out


class _Ctx:
    pass


def build_kernel(user, item, ncores=NCORES):
    nc = bacc.Bacc(None, target_bir_lowering=False, num_devices=ncores)
    cfgs = [user, item]

    ext = {}
    for c in cfgs:
        ext[c.name] = {
            "x": nc.declare_dram_parameter(f"x_{c.name}", [c.Np, 128], F32, isOutput=False),
            "W1": nc.declare_dram_parameter(f"W1_{c.name}", [c.d_in, HID], F32, isOutput=False),
            "W2": nc.declare_dram_parameter(f"W2_{c.name}", [HID, D_OUT], F32, isOutput=False),
            "gamma": nc.declare_dram_parameter(f"gamma_{c.name}", [D_OUT], F32, isOutput=False),
            "beta": nc.declare_dram_parameter(f"beta_{c.name}", [D_OUT], F32, isOutput=False),
            "out": nc.declare_dram_parameter(f"out_{c.name}", [c.n_out_pad, D_OUT], F32, isOutput=True),
        }
    ar_in = {c.name: nc.dram_tensor(f"ar_in_{c.name}", [128, 129], F32) for c in cfgs}
    ar_out = {c.name: nc.dram_tensor(f"ar_out_{c.name}", [128, 129], F32, addr_space="Shared")
              for c in cfgs}
    g = _Ctx()
    g.nc, g.rg = nc, [list(range(ncores))]

    with tile.TileContext(nc) as tc:
        g.tc = tc
        with tc.tile_pool(name="const", bufs=1) as constp:
            g.constp = constp
            ident_bf = constp.tile([128, 128], BF16)
            make_identity(nc, ident_bf[:])
            ident_f32 = constp.tile([128, 128], F32)
            make_identity(nc, ident_f32[:])
            ones_f32 = constp.tile([128, 1], F32)
            nc.gpsimd.memset(ones_f32[:], 1.0)
            ones_bf = constp.tile([128, 1], BF16)
            nc.gpsimd.memset(ones_bf[:], 1.0)
            eps_v = constp.tile([128, 1], F32)
            nc.gpsimd.memset(eps_v[:], EPS)
            zero_v = constp.tile([128, 1], F32)
            nc.gpsimd.memset(zero_v[:], 0.0)
            g.ident_bf, g.ident_f32 = ident_bf, ident_f32
            g.ones_f32, g.ones_bf = ones_f32, ones_bf
            g.eps_v, g.zero_v = eps_v, zero_v

            # ---- setup: Wc = W1 @ W2 per type ----
            g.wc_f32, g.wc_stack = {}, {}
            with (
                tc.tile_pool(name="wsetup", bufs=1) as wsp,
                tc.tile_pool(name="wps", bufs=1, space="PSUM") as wps,
            ):
                for c in cfgs:
                    d = c.d_in
                    w1 = wsp.tile([d, HID], F32, tag=f"w1_{c.name}")
                    nc.sync.dma_start(w1[:], ext[c.name]["W1"][:])
                    w2a = wsp.tile([128, D_OUT], F32, tag=f"w2a_{c.name}")
                    w2b = wsp.tile([128, D_OUT], F32, tag=f"w2b_{c.name}")
                    nc.sync.dma_start(w2a[:], ext[c.name]["W2"][0:128, :])
                    nc.sync.dma_start(w2b[:], ext[c.name]["W2"][128:256, :])
                    wc_ps = wps.tile([d, D_OUT], F32, tag="wc")
                    for h, w2h in enumerate((w2a, w2b)):
                        w1t_ps = wps.tile([128, d], F32, tag="w1t")
                        nc.tensor.transpose(w1t_ps[:], w1[:, h * 128:(h + 1) * 128],
                                            ident_f32[0:d, 0:d])
                        w1t = wsp.tile([128, d], F32, tag=f"w1t_{c.name}_{h}")
                        nc.vector.tensor_copy(w1t[:], w1t_ps[:])
                        nc.tensor.matmul(wc_ps[:], w1t[:], w2h[:],
                                         start=(h == 0), stop=(h == 1))
                    wf = constp.tile([d, D_OUT], F32, tag=f"wcf_{c.name}")
                    nc.vector.tensor_copy(wf[:], wc_ps[:])
                    g.wc_f32[c.name] = wf
                    ws = constp.tile([128, D_OUT], BF16, tag=f"wcs_{c.name}")
                    nc.scalar.copy(ws[0:d, :], wc_ps[:])
                    if c.paired:
                        nc.sync.dma_start(ws[64:128, :], ws[0:64, :])
                    g.wc_stack[c.name] = ws

            _phases(g, user, item, ext, ar_in, ar_out)

    nc.finalize()
    return nc


def _phases(g, user, item, ext, ar_in, ar_out):
    nc, tc = g.nc, g.tc
    with (
        tc.tile_pool(name="ut", bufs=1) as utp,
        tc.tile_pool(name="xchp", bufs=3) as xchp,
        tc.tile_pool(name="stgp", bufs=3) as stgp,
        tc.tile_pool(name="sbp", bufs=3) as sbp,
        tc.tile_pool(name="smp", bufs=1) as smp,
        tc.tile_pool(name="psA", bufs=4, space="PSUM") as psA,   # xT / Tback
        tc.tile_pool(name="psB", bufs=2, space="PSUM") as psB,   # uT
        tc.tile_pool(name="psG", bufs=1, space="PSUM") as psG,   # G+m, stats
    ):
        g.utp, g.xchp, g.stgp, g.sbp, g.smp = utp, xchp, stgp, sbp, smp
        g.psA, g.psB, g.psG = psA, psB, psG

        ut_item = utp.tile([128, item.Np], BF16)

        # ---- item p1 (G, m, uT store) ----
        _p1(g, item, ext[item.name], ut_item)
        _kick_ar(g, item, ar_in[item.name])
        nc.gpsimd.collective_compute("AllReduce", ALU.add, replica_groups=g.rg,
                                     ins=[ar_in[item.name][:]],
                                     outs=[ar_out[item.name][:]])

        # ---- user pA (G, m only; overlaps AR_item) ----
        _p1(g, user, ext[user.name], None)
        _kick_ar(g, user, ar_in[user.name])
        nc.gpsimd.collective_compute("AllReduce", ALU.add, replica_groups=g.rg,
                                     ins=[ar_in[user.name][:]],
                                     outs=[ar_out[user.name][:]])

        # ---- stats ----
        ab_item = _stats(g, item, ext[item.name], ar_out[item.name])
        ab_user = _stats(g, user, ext[user.name], ar_out[user.name])

        # ---- p2: item chunks interleaved with user-pB chunks ----
        ichunks = _chunks(item.Np)
        uchunks = _chunks(user.Np)
        ui = 0
        for k, (s0, qc) in enumerate(ichunks):
            _p2_item_chunk(g, item, ext[item.name], ut_item, ab_item, s0, qc)
            if (k >= P2_START_PURE and (k - P2_START_PURE) % P2_INTERLEAVE == 0
                    and ui < len(uchunks)):
                us0, uqc = uchunks[ui]
                _pb_user_chunk(g, user, ext[user.name], ab_user, us0, uqc)
                ui += 1
        while ui < len(uchunks):
            us0, uqc = uchunks[ui]
            _pb_user_chunk(g, user, ext[user.name], ab_user, us0, uqc)
            ui += 1


def _p1(g, c, ex, ut_store):
    """G + m accumulation; if ut_store is not None also transpose+uT+store.
    m: one ones^T @ [4-subtile slab] matmul per group accumulating into a
    [1,512] psum row (block-folded after the AllReduce kick).
    Two-group software-pipeline skew keeps the PE stream gapless (HAM warm).
    """
    nc = g.nc
    chunks = _chunks(c.Np)
    nsub = c.Np // 128
    n_groups = sum((qc + 3) // 4 for _, qc in chunks)
    full = ut_store is not None
    gm = g.psG.tile([128, 128], F32, tag="gbank")
    g.g_bank = getattr(g, "g_bank", {})
    g.g_bank[c.name] = gm
    g.m_bank = getattr(g, "m_bank", {})
    m_ps = g.psG.tile([1, 512], F32, tag="mps")
    g.m_bank[c.name] = m_ps

    def flush(pend):
        """downstream of one group: evict xT, uT matmul(s), store uT."""
        xt_ps, ns, blk0 = pend
        xt_sb = g.sbp.tile([128, 512], BF16, tag="xtsb")
        nc.vector.tensor_copy(xt_sb[:, 0:ns * 128], xt_ps[:, 0:ns * 128])
        if not c.paired:
            ut_ps = g.psB.tile([128, 512], F32, tag="utps")
            nc.tensor.matmul(ut_ps[:, 0:ns * 128], g.wc_stack[c.name][:],
                             xt_sb[:, 0:ns * 128], start=True, stop=True)
            nc.any.tensor_copy(ut_store[:, blk0 * 128:(blk0 + ns) * 128],
                               ut_ps[:, 0:ns * 128])
        else:
            for half in range(2):
                ut_ps = g.psB.tile([128, 512], F32, tag="utps")
                nc.tensor.matmul(
                    ut_ps[:, 0:ns * 128],
                    g.wc_stack[c.name][half * 64:(half + 1) * 64, :],
                    xt_sb[half * 64:(half + 1) * 64, 0:ns * 128],
                    start=True, stop=True)
                dst = ut_store[:, 2 * blk0 * 128:2 * (blk0 + ns) * 128]
                dst = dst.rearrange("p (n two f) -> p n two f",
                                    two=2, f=128)[:, :, half, :]
                src3 = ut_ps[:, 0:ns * 128].rearrange("p (n f) -> p n f", f=128)
                nc.any.tensor_copy(dst, src3)

    sub_i = 0
    grp_i = 0
    pend = []   # two-group software-pipeline skew
    for s0, qc in chunks:
        xch = g.xchp.tile([128, CHUNK_Q * 128], BF16, tag="xch")
        src = ex["x"][s0 * 128:(s0 + qc) * 128, :].rearrange(
            "(p q) d -> p (q d)", p=128)
        nc.gpsimd.dma_start(xch[:, 0:qc * 128], src)   # f32 -> bf16 cast
        for g0 in range(0, qc, 4):
            ns = min(4, qc - g0)
            slab = xch[:, g0 * 128:(g0 + ns) * 128]
            sls = [xch[:, (g0 + i) * 128:(g0 + i + 1) * 128] for i in range(ns)]
            if full:
                xt_ps = g.psA.tile([128, 512], BF16, tag="xtnat")
            else:
                xt_ps = None
            for i in range(ns):
                nc.tensor.matmul(gm[:], sls[i], sls[i],
                                 start=(sub_i == 0), stop=(sub_i == nsub - 1),
                                 skip_group_check=True)
                if full:
                    nc.tensor.transpose(xt_ps[:, i * 128:(i + 1) * 128],
                                        sls[i], g.ident_bf[:])
                sub_i += 1
            nc.tensor.matmul(m_ps[0:1, 0:ns * 128], g.ones_bf[:], slab,
                             start=(grp_i == 0), stop=(grp_i == n_groups - 1),
                             skip_group_check=True)
            grp_i += 1
            if not full:
                continue
            if len(pend) == 2:
                flush(pend.pop(0))
            pend.append((xt_ps, ns, s0 + g0))
    for p in pend:
        flush(p)
    assert sub_i == nsub and grp_i == n_groups


def _kick_ar(g, c, ar_in):
    nc = g.nc
    gm = g.g_bank[c.name]
    g_sb = g.smp.tile([128, 128], F32, tag=f"gsb_{c.name}")
    nc.vector.tensor_copy(g_sb[:], gm[:])
    nc.sync.dma_start(ar_in[:, 0:128], g_sb[:])
    # fold the [1,512] m row blocks and transpose to a per-partition column
    m_row4 = g.smp.tile([1, 512], F32, tag=f"mrow4_{c.name}")
    nc.vector.tensor_copy(m_row4[:], g.m_bank[c.name][:])
    m_row = g.smp.tile([1, 128], F32, tag=f"mrow_{c.name}")
    nc.vector.tensor_tensor(m_row[:], m_row4[:, 0:128], m_row4[:, 128:256], ALU.add)
    nc.vector.tensor_tensor(m_row[:], m_row[:], m_row4[:, 256:384], ALU.add)
    nc.vector.tensor_tensor(m_row[:], m_row[:], m_row4[:, 384:512], ALU.add)
    mt_ps = g.psG.tile([128, 4], F32, tag="mps")
    nc.tensor.transpose(mt_ps[:, 0:1], m_row[:], g.ident_f32[0:1, 0:1])
    m_col = g.smp.tile([128, 1], F32, tag=f"mcol_{c.name}")
    nc.vector.tensor_copy(m_col[:], mt_ps[:, 0:1])
    nc.sync.dma_start(ar_in[:, 128:129], m_col[:])


def _stats(g, c, ex, ar_out):
    """AllReduced [G|m] -> (a_vec, b_vec) [128,1] f32."""
    nc, d = g.nc, c.d_in
    smp, psG = g.smp, g.psG
    name = c.name
    ar_sb = smp.tile([128, 129], F32, tag=f"arsb_{name}")
    nc.sync.dma_start(ar_sb[:], ar_out[:])

    if c.paired:
        tmp = smp.tile([64, 65], F32, tag=f"fold_{name}")
        nc.sync.dma_start(tmp[:], ar_sb[64:128, 64:129])   # cross-partition
        g_eff = smp.tile([64, 64], F32, tag=f"geff_{name}")
        nc.vector.tensor_tensor(g_eff[:], ar_sb[0:64, 0:64], tmp[:, 0:64], ALU.add)
        m_eff = smp.tile([64, 1], F32, tag=f"meff_{name}")
        nc.vector.tensor_tensor(m_eff[:], ar_sb[0:64, 128:129], tmp[:, 64:65],
                                ALU.add)
        g_ap, m_ap = g_eff[:], m_eff[:]
    else:
        g_ap, m_ap = ar_sb[:, 0:128], ar_sb[:, 128:129]

    wc = g.wc_f32[name]
    t1_ps = psG.tile([d, D_OUT], F32, tag="mps")
    nc.tensor.matmul(t1_ps[:], g_ap, wc[:], start=True, stop=True)
    t1 = smp.tile([d, D_OUT], F32, tag=f"t1_{name}")
    nc.vector.tensor_copy(t1[:], t1_ps[:])
    t2 = smp.tile([d, D_OUT], F32, tag=f"t2_{name}")
    nc.vector.tensor_tensor(t2[:], t1[:], wc[:], ALU.mult)
    e2_ps = psG.tile([1, D_OUT], F32, tag="mps")
    nc.tensor.matmul(e2_ps[:], g.ones_f32[0:d, :], t2[:], start=True, stop=True)
    e2_sb = smp.tile([1, D_OUT], F32, tag=f"e2sb_{name}")
    nc.vector.tensor_copy(e2_sb[:], e2_ps[:])
    s_ps = psG.tile([1, D_OUT], F32, tag="mps")
    nc.tensor.matmul(s_ps[:], m_ap, wc[:], start=True, stop=True)
    s_sb = smp.tile([1, D_OUT], F32, tag=f"ssb_{name}")
    nc.vector.tensor_copy(s_sb[:], s_ps[:])
    # assemble 4 rows on partitions 0..3 via DMA (engines cannot cross
    # partitions), then one PE transpose to get per-partition columns
    rowmat = smp.tile([4, 128], F32, tag=f"rowmat_{name}")
    nc.sync.dma_start(rowmat[0:1, :], e2_sb[:])
    nc.sync.dma_start(rowmat[1:2, :], s_sb[:])
    nc.sync.dma_start(rowmat[2:3, :], ex["gamma"][:].rearrange("(o f) -> o f", o=1))
    nc.sync.dma_start(rowmat[3:4, :], ex["beta"][:].rearrange("(o f) -> o f", o=1))
    cols_ps = psG.tile([128, 4], F32, tag="mps")
    nc.tensor.transpose(cols_ps[:], rowmat[:], g.ident_f32[0:4, 0:4])
    cols = smp.tile([128, 4], F32, tag=f"cols_{name}")
    nc.vector.tensor_copy(cols[:], cols_ps[:])

    inv_n = 1.0 / float(c.N_total)
    mean = smp.tile([128, 1], F32, tag=f"mean_{name}")
    nc.vector.tensor_scalar(mean[:], cols[:, 1:2], inv_n, None, ALU.mult)
    msq = smp.tile([128, 1], F32, tag=f"msq_{name}")
    nc.vector.tensor_tensor(msq[:], mean[:], mean[:], ALU.mult)
    e2n = smp.tile([128, 1], F32, tag=f"e2n_{name}")
    nc.vector.tensor_scalar(e2n[:], cols[:, 0:1], inv_n, None, ALU.mult)
    var = smp.tile([128, 1], F32, tag=f"var_{name}")
    nc.vector.tensor_tensor(var[:], e2n[:], msq[:], ALU.subtract)
    lnv = smp.tile([128, 1], F32, tag=f"lnv_{name}")
    nc.scalar.activation(lnv[:], var[:], AFT.Ln, bias=g.eps_v[:], scale=1.0)
    rstd = smp.tile([128, 1], F32, tag=f"rstd_{name}")
    nc.scalar.activation(rstd[:], lnv[:], AFT.Exp, bias=g.zero_v[:], scale=-0.5)
    a_vec = smp.tile([128, 1], F32, tag=f"avec_{name}")
    nc.vector.tensor_tensor(a_vec[:], cols[:, 2:3], rstd[:], ALU.mult)
    ma = smp.tile([128, 1], F32, tag=f"ma_{name}")
    nc.vector.tensor_tensor(ma[:], mean[:], a_vec[:], ALU.mult)
    b_vec = smp.tile([128, 1], F32, tag=f"bvec_{name}")
    nc.vector.tensor_tensor(b_vec[:], cols[:, 3:4], ma[:], ALU.subtract)
    return a_vec, b_vec, {"var": var, "e2n": e2n, "rstd": rstd, "mean": mean,
                          "cols": cols, "arsb": ar_sb, "t2": t2,
                          "geff": g_ap if c.paired else None}


def _p2_item_chunk(g, c, ex, ut_store, ab, s0, qc):
    nc = g.nc
    a_vec, b_vec = ab[0], ab[1]
    zgs = []
    # emit the whole chunk's Lrelu ops first so ScalarE runs ahead of PE
    for z0 in range(0, qc, 8):
        zn = min(8, qc - z0)
        zw = zn * 128
        lr = g.sbp.tile([128, 1024], BF16, tag="lrelu")
        nc.scalar.activation(lr[:, 0:zw], ut_store[:, (s0 + z0) * 128:
                                                    (s0 + z0) * 128 + zw],
                             AFT.Lrelu, bias=b_vec[:], scale=a_vec[:],
                             alpha=NEG_SLOPE)
        zgs.append((z0, zn, zw, lr))
    for z0, zn, zw, lr in zgs:
        stg = g.stgp.tile([128, 1024], F32, tag="stg")
        for t0 in range(0, zn, 4):
            tn = min(4, zn - t0)
            nat_ps = g.psA.tile([128, 512], BF16, tag="xtnat")
            for i in range(tn):
                nc.tensor.transpose(nat_ps[:, i * 128:(i + 1) * 128],
                                    lr[:, (t0 + i) * 128:(t0 + i + 1) * 128],
                                    g.ident_bf[:])
            nc.any.tensor_copy(stg[:, t0 * 128:(t0 + tn) * 128],
                               nat_ps[:, 0:tn * 128])
        dst = ex["out"][s0 * 128:(s0 + qc) * 128, :].rearrange(
            "(p q) e -> p (q e)", p=128)[:, z0 * 128:z0 * 128 + zw]
        nc.sync.dma_start(dst, stg[:, 0:zw])


def _pb_user_chunk(g, c, ex, ab, s0, qc):
    """user pass B: re-read x, recompute uT, Lrelu from psum, transpose back."""
    nc = g.nc
    a_vec, b_vec = ab[0], ab[1]
    xch = g.xchp.tile([128, CHUNK_Q * 128], BF16, tag="xch")
    src = ex["x"][s0 * 128:(s0 + qc) * 128, :].rearrange("(p q) d -> p (q d)", p=128)
    nc.gpsimd.dma_start(xch[:, 0:qc * 128], src)
    out_rr = ex["out"][s0 * 256:(s0 + qc) * 256, :].rearrange(
        "(p q) e -> p (q e)", p=128)

    def flush(pend):
        g0, ns, xt_ps = pend
        xt_sb = g.sbp.tile([128, 512], BF16, tag="xtsb")
        nc.vector.tensor_copy(xt_sb[:, 0:ns * 128], xt_ps[:, 0:ns * 128])
        stg = g.stgp.tile([128, 1024], F32, tag="stg")
        for half in range(2):
            ut_ps = g.psB.tile([128, 512], F32, tag="utps")
            nc.tensor.matmul(ut_ps[:, 0:ns * 128],
                             g.wc_stack[c.name][half * 64:(half + 1) * 64, :],
                             xt_sb[half * 64:(half + 1) * 64, 0:ns * 128],
                             start=True, stop=True)
            lr = g.sbp.tile([128, 512], BF16, tag="lrelu_u")
            nc.scalar.activation(lr[:, 0:ns * 128], ut_ps[:, 0:ns * 128],
                                 AFT.Lrelu, bias=b_vec[:], scale=a_vec[:],
                                 alpha=NEG_SLOPE)
            nat_ps = g.psA.tile([128, 512], BF16, tag="xtnat")
            for i in range(ns):
                nc.tensor.transpose(nat_ps[:, i * 128:(i + 1) * 128],
                                    lr[:, i * 128:(i + 1) * 128], g.ident_bf[:])
            # real blocks 2i+half within this group -> strided staging cols
            dst = stg[:, 0:2 * ns * 128].rearrange(
                "p (n two f) -> p n two f", two=2, f=128)[:, :, half, :]
            nc.any.tensor_copy(dst, nat_ps[:, 0:ns * 128].rearrange(
                "p (n f) -> p n f", f=128))
        dcols = 2 * ns * 128
        nc.sync.dma_start(out_rr[:, 2 * g0 * 128:2 * g0 * 128 + dcols],
                          stg[:, 0:dcols])

    pending = None
    for g0 in range(0, qc, 4):
        ns = min(4, qc - g0)
        xt_ps = g.psA.tile([128, 512], BF16, tag="xtnat")
        for i in range(ns):
            nc.tensor.transpose(xt_ps[:, i * 128:(i + 1) * 128],
                                xch[:, (g0 + i) * 128:(g0 + i + 1) * 128],
                                g.ident_bf[:])
        if pending is not None:
            flush(pending)
        pending = (g0, ns, xt_ps)
    if pending is not None:
        flush(pending)


# ---------------------------------------------------------------------------
_BUILT = {}


def _get_built():
    if "full" not in _BUILT:
        user = TypeCfg("user", N_USER // NCORES, 64, N_USER)
        item = TypeCfg("item", N_ITEM // NCORES, 128, N_ITEM)
        _BUILT["full"] = (build_kernel(user, item), user, item)
    return _BUILT["full"]


def kernel(x_user, x_item,
           W1_user=None, b1_user=None, W1_item=None, b1_item=None,
           W2_user=None, b2_user=None, W2_item=None, b2_item=None,
           gamma_user=None, beta_user=None, gamma_item=None, beta_item=None,
           _trace=False):
    nc, ucfg, icfg = _get_built()

    def prep(x, cfg):
        x = np.ascontiguousarray(np.asarray(x, np.float32))
        n = x.shape[0] // NCORES
        shards = []
        for i in range(NCORES):
            s = x[i * n:(i + 1) * n].reshape(-1, 128)
            pad = cfg.Np - s.shape[0]
            if pad:
                s = np.concatenate([s, np.zeros((pad, 128), np.float32)], 0)
            shards.append(s)
        return shards

    xu = prep(x_user, ucfg)
    xi = prep(x_item, icfg)
    common = {
        "W1_user": np.asarray(W1_user, np.float32),
        "W2_user": np.asarray(W2_user, np.float32),
        "gamma_user": np.asarray(gamma_user, np.float32),
        "beta_user": np.asarray(beta_user, np.float32),
        "W1_item": np.asarray(W1_item, np.float32),
        "W2_item": np.asarray(W2_item, np.float32),
        "gamma_item": np.asarray(gamma_item, np.float32),
        "beta_item": np.asarray(beta_item, np.float32),
    }
    in_maps = [dict(common, x_user=xu[i], x_item=xi[i]) for i in range(NCORES)]
    res = run_bass_kernel_spmd(nc, in_maps, list(range(NCORES)), trace=_trace)
    nu, ni = N_USER // NCORES, N_ITEM // NCORES
    out_user = np.concatenate(
        [res.results[i]["out_user"][:nu] for i in range(NCORES)], 0)
    out_item = np.concatenate(
        [res.results[i]["out_item"][:ni] for i in range(NCORES)], 0)
    if _trace:
        kernel.last_exec_time_ns = res.exec_time_ns
    return (out_user, out_item)
